# revision 1
# baseline (speedup 1.0000x reference)
import sys, os
sys.path.insert(0, '/opt/trn_rl_repo')
import numpy as np
import concourse.bass as bass
import concourse.tile as tile
from concourse import bacc, mybir
from concourse.bass_utils import run_bass_kernel_spmd
from concourse.masks import make_identity

dt = mybir.dt
f32, f32r, bf16 = dt.float32, dt.float32r, dt.bfloat16

B, S, D, DFF, H, V, L = 2, 1024, 1024, 4096, 16, 32000, 8
DH = D // H          # 64
NC = 8               # cores
TOK = (B * S) // NC  # 256 tokens per core
NTB = TOK // 128     # 2 token blocks per core
NDC = D // 128       # 8 d-chunks
NKC = (S) // 128     # 8 key chunks per batch
NFFC = DFF // 128    # 32
VSH = V // NC        # 4000 vocab per core
VPAD = 4096          # padded vocab slice
EPS = 1e-5


def _pe():
    pos = np.arange(S, dtype=np.float32)[:, None]
    div = np.exp(np.arange(0, D, 2, dtype=np.float32) * (-np.log(10000.0) / D))
    pe = np.zeros((S, D), dtype=np.float32)
    pe[:, 0::2] = np.sin(pos * div)
    pe[:, 1::2] = np.cos(pos * div)
    return pe


def build_program(ln1_triv, ln2_triv, b2_triv, lnf_triv):
    nc = bacc.Bacc("TRN2", target_bir_lowering=False, debug=False,
                   enable_asserts=False, num_devices=NC)

    # ---- DRAM inputs (shared weights) ----
    wq_d = nc.dram_tensor("wq", [L, D, D], bf16, kind="ExternalInput").ap()
    wk_d = nc.dram_tensor("wk", [L, D, D], bf16, kind="ExternalInput").ap()
    wv_d = nc.dram_tensor("wv", [L, D, D], bf16, kind="ExternalInput").ap()
    wo_d = nc.dram_tensor("wo", [L, D, D], bf16, kind="ExternalInput").ap()
    w1_d = nc.dram_tensor("w1", [L, D, DFF], bf16, kind="ExternalInput").ap()
    w2_d = nc.dram_tensor("w2", [L, DFF, D], bf16, kind="ExternalInput").ap()
    b1_d = nc.dram_tensor("b1", [L, DFF], f32, kind="ExternalInput").ap()
    # ---- per-core inputs ----
    embx_d = nc.dram_tensor("embx", [TOK, D], f32, kind="ExternalInput").ap()
    pe_d = nc.dram_tensor("pe", [TOK, D], f32, kind="ExternalInput").ap()
    mt_d = nc.dram_tensor("mt", [NKC, 128, TOK], bf16, kind="ExternalInput").ap()
    pw_d = nc.dram_tensor("pw", [D, VPAD], bf16, kind="ExternalInput").ap()
    pb_d = nc.dram_tensor("pb", [VPAD], f32, kind="ExternalInput").ap()
    # optional non-trivial affine params (pre-broadcast on host)
    if not (ln1_triv and ln2_triv):
        lngb_d = nc.dram_tensor("lngb", [L, 4, 128, D], f32, kind="ExternalInput").ap()
    if not b2_triv:
        b2b_d = nc.dram_tensor("b2b", [L, 128, D], f32, kind="ExternalInput").ap()
    if not lnf_triv:
        fgb_d = nc.dram_tensor("fgb", [2, 128, D], f32, kind="ExternalInput").ap()
    # ---- output ----
    out_d = nc.dram_tensor("logits", [VPAD, B * S], f32, kind="ExternalOutput").ap()

    from contextlib import ExitStack
    with tile.TileContext(nc) as tc:
        with ExitStack() as ctx:
            cpool = ctx.enter_context(tc.tile_pool(name="const", bufs=1))
            rpool = ctx.enter_context(tc.tile_pool(name="resid", bufs=1))
            atpool = ctx.enter_context(tc.tile_pool(name="aT", bufs=2))
            qkvpool = ctx.enter_context(tc.tile_pool(name="qkv", bufs=1))
            wpool = ctx.enter_context(tc.tile_pool(name="wch", bufs=4))
            w2pool = ctx.enter_context(tc.tile_pool(name="w2c", bufs=3))
            utpool = ctx.enter_context(tc.tile_pool(name="ut", bufs=32))
            htpool = ctx.enter_context(tc.tile_pool(name="ht", bufs=2))
            apool = ctx.enter_context(tc.tile_pool(name="att", bufs=6))
            alnpool = ctx.enter_context(tc.tile_pool(name="aln", bufs=2))
            spool = ctx.enter_context(tc.tile_pool(name="small", bufs=4))
            recpool = ctx.enter_context(tc.tile_pool(name="recp", bufs=2))
            psmm = ctx.enter_context(tc.tile_pool(name="ps_mm", bufs=4, space="PSUM"))
            pssum = ctx.enter_context(tc.tile_pool(name="ps_sum", bufs=2, space="PSUM"))
            pso = ctx.enter_context(tc.tile_pool(name="ps_o", bufs=2, space="PSUM"))
            dpool = ctx.enter_context(tc.tile_pool(name="dram", bufs=1, space="DRAM"))

            # ---------------- constants ----------------
            ident = cpool.tile([128, 128], bf16, tag="ident")
            make_identity(nc, ident[:])
            ones = cpool.tile([128, 64], bf16, tag="ones")
            nc.vector.memset(ones[:], 1.0)
            epst = cpool.tile([128, 1], f32, tag="eps")
            nc.vector.memset(epst[:], EPS)
            mtt = cpool.tile([128, NKC, TOK], bf16, tag="mt")
            nc.sync.dma_start(out=mtt[:], in_=mt_d.rearrange("kc p q -> p kc q"))
            b1t = cpool.tile([128, L, NFFC], f32, tag="b1")
            nc.sync.dma_start(out=b1t[:], in_=b1_d.rearrange("l (fc p) -> p l fc", p=128))
            pbt = cpool.tile([128, VPAD // 128], f32, tag="pb")
            nc.sync.dma_start(out=pbt[:], in_=pb_d.rearrange("(vc p) -> p vc", p=128))
            if not (ln1_triv and ln2_triv):
                lngb = cpool.tile([128, L, 4, D], f32, tag="lngb")
                nc.sync.dma_start(out=lngb[:], in_=lngb_d.rearrange("l f p d -> p l f d"))
            if not b2_triv:
                b2b = cpool.tile([128, L, D], f32, tag="b2b")
                nc.sync.dma_start(out=b2b[:], in_=b2b_d.rearrange("l p d -> p l d"))
            if not lnf_triv:
                fgb = cpool.tile([128, 2, D], f32, tag="fgb")
                nc.sync.dma_start(out=fgb[:], in_=fgb_d.rearrange("f p d -> p f d"))

            # collective DRAM buffers
            kv_in = dpool.tile([2 * TOK * D], bf16, tag="kv_in")
            kv_out = dpool.tile([4, 2 * TOK * D], bf16, tag="kv_out")
            ag2_in = dpool.tile([TOK * D], bf16, tag="ag2_in")
            ag2_out = dpool.tile([NC, TOK * D], bf16, tag="ag2_out", addr_space="Shared")

            # ---------------- embedding ----------------
            h = [rpool.tile([128, D], f32, tag=f"h{tb}", name=f"h{tb}") for tb in range(NTB)]
            for tb in range(NTB):
                et = alnpool.tile([128, D], f32, tag="aln")
                pt = alnpool.tile([128, D], f32, tag="aln")
                nc.sync.dma_start(out=et, in_=embx_d[tb * 128:(tb + 1) * 128, :])
                nc.sync.dma_start(out=pt, in_=pe_d[tb * 128:(tb + 1) * 128, :])
                nc.vector.tensor_scalar(out=et[:], in0=et[:], scalar1=float(np.sqrt(D)),
                                        scalar2=None, op0=mybir.AluOpType.mult)
                nc.vector.tensor_add(h[tb][:], et[:], pt[:])

            def layer_norm(src_tiles, dst_tag, gb=None):
                """LN along free dim of token-major tiles; returns new tiles."""
                outs = []
                for tb in range(NTB):
                    st = spool.tile([128, 2, 6], f32, tag="bnst")
                    xin = src_tiles[tb][:].rearrange("p (g d) -> p g d", g=2)
                    for g in range(2):
                        nc.vector.bn_stats(out=st[:, g, :], in_=xin[:, g, :])
                    mv = spool.tile([128, 2], f32, tag="bnmv")
                    nc.vector.bn_aggr(out=mv[:], in_=st[:])
                    std = spool.tile([128, 1], f32, tag="bnsd")
                    nc.scalar.activation(out=std[:], in_=mv[:, 1:2],
                                         func=mybir.ActivationFunctionType.Sqrt,
                                         bias=epst[:], scale=1.0)
                    nc.vector.reciprocal(out=std[:], in_=std[:])
                    at = alnpool.tile([128, D], bf16, tag=dst_tag)
                    nc.vector.tensor_scalar(out=at[:], in0=src_tiles[tb][:],
                                            scalar1=mv[:, 0:1], scalar2=std[:],
                                            op0=mybir.AluOpType.subtract,
                                            op1=mybir.AluOpType.mult)
                    if gb is not None:
                        gt, bt = gb
                        nc.vector.tensor_mul(at[:], at[:], gt)
                        nc.vector.tensor_add(at[:], at[:], bt)
                    outs.append(at)
                return outs

            def transpose_to_aT(src_tiles):
                """token-major [128, D] x NTB -> aT [128, NDC, TOK] (fp32r)."""
                at = atpool.tile([128, NDC, TOK], bf16, tag="aT")
                for tb in range(NTB):
                    for dc in range(NDC):
                        ps = psmm.tile([128, 128], bf16, tag="mm")
                        nc.tensor.transpose(ps[:], src_tiles[tb][:, dc * 128:(dc + 1) * 128], ident[:])
                        nc.vector.tensor_copy(out=at[:, dc, tb * 128:(tb + 1) * 128], in_=ps[:])
                return at

            # ---------------- layers ----------------
            for li in range(L):
                ln1_gb = None
                if not ln1_triv:
                    ln1_gb = (lngb[:, li, 0, :], lngb[:, li, 1, :])
                aln = layer_norm(h, "aln", ln1_gb)
                aT = transpose_to_aT(aln)

                # --- K projection first (feeds AllGather ASAP) ---
                qT = qkvpool.tile([128, NDC, TOK], bf16, tag="qT")
                kloc = qkvpool.tile([128, NDC, TOK], bf16, tag="kloc")
                wsrc = wk_d[li].rearrange("(dc p) j -> p dc j", p=128)
                for cg in range(2):
                    wc = wpool.tile([128, NDC, 512], bf16, tag="wch")
                    nc.sync.dma_start(out=wc, in_=wsrc[:, :, cg * 512:(cg + 1) * 512])
                    for oc in range(4):
                        jc = cg * 4 + oc
                        ps = psmm.tile([128, TOK], f32, tag="mm")
                        for dc in range(NDC):
                            nc.tensor.matmul(ps[:], wc[:, dc, oc * 128:(oc + 1) * 128],
                                             aT[:, dc, :], start=(dc == 0), stop=(dc == NDC - 1))
                        nc.vector.tensor_copy(out=kloc[:, jc, :], in_=ps[:])

                # --- V projection: token-major out [t, j] ---
                vloc = [qkvpool.tile([128, D], bf16, tag=f"vloc{tb}", name=f"vloc{tb}") for tb in range(NTB)]
                wsrc = wv_d[li].rearrange("(dc p) j -> p dc j", p=128)
                for cg in range(2):
                    wc = wpool.tile([128, NDC, 512], bf16, tag="wch")
                    nc.sync.dma_start(out=wc, in_=wsrc[:, :, cg * 512:(cg + 1) * 512])
                    for tb in range(NTB):
                        ps = psmm.tile([128, 512], f32, tag="mm")
                        for dc in range(NDC):
                            nc.tensor.matmul(ps[:], aT[:, dc, tb * 128:(tb + 1) * 128],
                                             wc[:, dc, :], start=(dc == 0), stop=(dc == NDC - 1))
                        nc.vector.tensor_copy(out=vloc[tb][:, cg * 512:(cg + 1) * 512], in_=ps[:])

                # --- pack K/V and AllGather within batch group ---
                nc.sync.dma_start(
                    out=kv_in[0:TOK * D].rearrange("(jc p k) -> p jc k", p=128, k=TOK),
                    in_=kloc[:])
                for tb in range(NTB):
                    nc.sync.dma_start(
                        out=kv_in[TOK * D:2 * TOK * D].rearrange(
                            "(tb p j) -> p tb j", p=128, j=D)[:, tb, :],
                        in_=vloc[tb][:])
                nc.gpsimd.collective_compute(
                    "AllGather", mybir.AluOpType.bypass,
                    replica_groups=[[0, 1, 2, 3], [4, 5, 6, 7]],
                    ins=[kv_in[:].opt()], outs=[kv_out[:].opt()])

                # --- Q projection (overlaps the AllGather) ---
                wsrc = wq_d[li].rearrange("(dc p) j -> p dc j", p=128)
                for cg in range(2):
                    wc = wpool.tile([128, NDC, 512], bf16, tag="wch")
                    nc.sync.dma_start(out=wc, in_=wsrc[:, :, cg * 512:(cg + 1) * 512])
                    for oc in range(4):
                        jc = cg * 4 + oc
                        ps = psmm.tile([128, TOK], f32, tag="mm")
                        for dc in range(NDC):
                            nc.tensor.matmul(ps[:], wc[:, dc, oc * 128:(oc + 1) * 128],
                                             aT[:, dc, :], start=(dc == 0), stop=(dc == NDC - 1))
                        nc.vector.tensor_copy(out=qT[:, jc, :], in_=ps[:])

                # --- attention ---
                oall = qkvpool.tile([128, NDC, TOK], bf16, tag="oall")
                kv_k = kv_out[:, 0:TOK * D].rearrange("s (jc p k) -> p s jc k", jc=NDC, p=128, k=TOK)
                kv_v = kv_out[:, TOK * D:2 * TOK * D].rearrange(
                    "s (tb p j) -> p s tb j", tb=NTB, p=128, j=D)
                for hp in range(H // 2):
                    ktile = htpool.tile([128, 4, TOK], bf16, tag="kt")
                    nc.sync.dma_start(out=ktile, in_=kv_k[:, :, hp, :])
                    vtile = htpool.tile([128, 4, NTB, 128], bf16, tag="vt")
                    for k2 in range(NTB):
                        nc.sync.dma_start(out=vtile[:, :, k2, :],
                                          in_=kv_v[:, :, k2, hp * 128:(hp + 1) * 128])
                    for hh in range(2):
                        hb = hh * 64
                        ps_o = pso.tile([64, TOK], f32, tag="o")
                        ps_sum = pssum.tile([1, TOK], f32, tag="sum")
                        for kc in range(NKC):
                            ps_s = psmm.tile([128, TOK], f32, tag="mm")
                            nc.tensor.matmul(
                                ps_s[:],
                                ktile[hb:hb + 64, kc // 2, (kc % 2) * 128:(kc % 2) * 128 + 128],
                                qT[hb:hb + 64, hp, :], start=True, stop=True)
                            et = apool.tile([128, TOK], bf16, tag="att")
                            nc.scalar.activation(out=et[:], in_=ps_s[:],
                                                 func=mybir.ActivationFunctionType.Exp,
                                                 scale=float(1.0 / np.sqrt(DH)))
                            at2 = apool.tile([128, TOK], bf16, tag="att")
                            nc.vector.tensor_mul(at2[:], et[:], mtt[:, kc, :])
                            nc.tensor.matmul(ps_sum[:], ones[:, 0:1], at2[:],
                                             start=(kc == 0), stop=(kc == NKC - 1))
                            nc.tensor.matmul(ps_o[:], vtile[:, kc // 2, kc % 2, hb:hb + 64],
                                             at2[:], start=(kc == 0), stop=(kc == NKC - 1))
                        ssb = recpool.tile([1, TOK], bf16, tag="rec")
                        with nc.allow_low_precision(reason="softmax denom bcast via matmul"):
                            nc.vector.tensor_copy(out=ssb[:], in_=ps_sum[:])
                        ps_rb = pso.tile([64, TOK], f32, tag="o")
                        nc.tensor.matmul(ps_rb[:], ones[0:1, 0:64], ssb[:], start=True, stop=True)
                        rb = apool.tile([128, TOK], f32, tag="attf")
                        nc.vector.reciprocal(out=rb[0:64, :], in_=ps_rb[:])
                        nc.vector.tensor_mul(oall[hb:hb + 64, hp, :], ps_o[:], rb[0:64, :])

                # --- Wo projection (token-major out) + residual ---
                wsrc = wo_d[li].rearrange("(jc p) o -> p jc o", p=128)
                for cg in range(2):
                    wc = wpool.tile([128, NDC, 512], bf16, tag="wch")
                    nc.sync.dma_start(out=wc, in_=wsrc[:, :, cg * 512:(cg + 1) * 512])
                    for tb in range(NTB):
                        ps = psmm.tile([128, 512], f32, tag="mm")
                        for jc in range(NDC):
                            nc.tensor.matmul(ps[:], oall[:, jc, tb * 128:(tb + 1) * 128],
                                             wc[:, jc, :], start=(jc == 0), stop=(jc == NDC - 1))
                        nc.vector.tensor_add(h[tb][:, cg * 512:(cg + 1) * 512],
                                             h[tb][:, cg * 512:(cg + 1) * 512], ps[:])

                # --- FFN ---
                ln2_gb = None
                if not ln2_triv:
                    ln2_gb = (lngb[:, li, 2, :], lngb[:, li, 3, :])
                f_ln = layer_norm(h, "aln", ln2_gb)
                fT = transpose_to_aT(f_ln)

                uts = []
                wsrc = w1_d[li].rearrange("(dc p) j -> p dc j", p=128)
                for cg in range(NFFC // 4):
                    wc = wpool.tile([128, NDC, 512], bf16, tag="wch")
                    nc.sync.dma_start(out=wc, in_=wsrc[:, :, cg * 512:(cg + 1) * 512])
                    for oc in range(4):
                        fc = cg * 4 + oc
                        ps = psmm.tile([128, TOK], f32, tag="mm")
                        for dc in range(NDC):
                            nc.tensor.matmul(ps[:], wc[:, dc, oc * 128:(oc + 1) * 128],
                                             fT[:, dc, :], start=(dc == 0), stop=(dc == NDC - 1))
                        ut = utpool.tile([128, TOK], bf16, tag="ut")
                        nc.vector.tensor_scalar(out=ut[:], in0=ps[:],
                                                scalar1=b1t[:, li, fc:fc + 1], scalar2=0.0,
                                                op0=mybir.AluOpType.add,
                                                op1=mybir.AluOpType.max)
                        uts.append(ut)

                # --- W2: 4 psum chains (tb x og), accumulate over ffc ---
                wsrc = w2_d[li].rearrange("(fc p) o -> p fc o", p=128)
                chains = {}
                for tb in range(NTB):
                    for og in range(2):
                        chains[(tb, og)] = psmm.tile([128, 512], f32, tag="mm", name=f"w2ch{tb}{og}")
                for fcg in range(NFFC // 2):
                    wc = w2pool.tile([128, 2, D], bf16, tag="w2c")
                    nc.sync.dma_start(out=wc, in_=wsrc[:, fcg * 2:fcg * 2 + 2, :])
                    for f2 in range(2):
                        fc = fcg * 2 + f2
                        for tb in range(NTB):
                            for og in range(2):
                                nc.tensor.matmul(chains[(tb, og)][:],
                                                 uts[fc][:, tb * 128:(tb + 1) * 128],
                                                 wc[:, f2, og * 512:(og + 1) * 512],
                                                 start=(fc == 0), stop=(fc == NFFC - 1))
                for tb in range(NTB):
                    for og in range(2):
                        nc.vector.tensor_add(h[tb][:, og * 512:(og + 1) * 512],
                                             h[tb][:, og * 512:(og + 1) * 512],
                                             chains[(tb, og)][:])
                    if not b2_triv:
                        nc.vector.tensor_add(h[tb][:], h[tb][:], b2b[:, li, :])

            # ---------------- final LN + AllGather + projection ----------------
            fin_gb = None if lnf_triv else (fgb[:, 0, :], fgb[:, 1, :])
            fin = layer_norm(h, "aln", fin_gb)
            finT = transpose_to_aT(fin)
            nc.sync.dma_start(
                out=ag2_in[:].rearrange("(jc p t) -> p jc t", p=128, t=TOK),
                in_=finT[:])
            nc.gpsimd.collective_compute(
                "AllGather", mybir.AluOpType.bypass,
                replica_groups=[[0, 1, 2, 3, 4, 5, 6, 7]],
                ins=[ag2_in[:].opt()], outs=[ag2_out[:].opt()])

            hfull = ag2_out[:].rearrange("s (jc p t) -> p s jc t", jc=NDC, p=128, t=TOK)
            pwsrc = pw_d.rearrange("(dc p) v -> p dc v", p=128)
            for wc_i in range(VPAD // 512):
                pwc = wpool.tile([128, NDC, 512], bf16, tag="wch")
                nc.sync.dma_start(out=pwc, in_=pwsrc[:, :, wc_i * 512:(wc_i + 1) * 512])
                for tb in range(NC):
                    htl = htpool.tile([128, NDC, TOK], bf16, tag="kt")
                    nc.sync.dma_start(out=htl, in_=hfull[:, tb, :, :])
                    for vc in range(4):
                        vg = wc_i * 4 + vc
                        ps = psmm.tile([128, TOK], f32, tag="mm")
                        for dc in range(NDC):
                            nc.tensor.matmul(ps[:], pwc[:, dc, vc * 128:(vc + 1) * 128],
                                             htl[:, dc, :], start=(dc == 0), stop=(dc == NDC - 1))
                        lsb = apool.tile([128, TOK], f32, tag="attf")
                        nc.vector.tensor_scalar(out=lsb[:], in0=ps[:],
                                                scalar1=pbt[:, vg:vg + 1], scalar2=None,
                                                op0=mybir.AluOpType.add)
                        nc.sync.dma_start(
                            out=out_d[vg * 128:(vg + 1) * 128, tb * TOK:(tb + 1) * TOK],
                            in_=lsb[:])
    nc.compile()
    return nc


def kernel(**inputs):
    x = np.asarray(inputs["x"])
    mask = np.asarray(inputs["mask"])
    emb = np.asarray(inputs["emb"], dtype=np.float32)

    ln1_g = np.asarray(inputs["ln1_g"], dtype=np.float32)
    ln1_b = np.asarray(inputs["ln1_b"], dtype=np.float32)
    ln2_g = np.asarray(inputs["ln2_g"], dtype=np.float32)
    ln2_b = np.asarray(inputs["ln2_b"], dtype=np.float32)
    lnf_g = np.asarray(inputs["lnf_g"], dtype=np.float32)
    lnf_b = np.asarray(inputs["lnf_b"], dtype=np.float32)
    b2 = np.asarray(inputs["b2"], dtype=np.float32)

    ln1_triv = bool(np.all(ln1_g == 1) and np.all(ln1_b == 0))
    ln2_triv = bool(np.all(ln2_g == 1) and np.all(ln2_b == 0))
    lnf_triv = bool(np.all(lnf_g == 1) and np.all(lnf_b == 0))
    b2_triv = bool(np.all(b2 == 0))

    nc = build_program(ln1_triv, ln2_triv, b2_triv, lnf_triv)

    pe_full = _pe()
    ids = np.asarray(x).reshape(B * S)
    m2d = np.asarray(mask[0, 0], dtype=np.float32)  # [S(q), S(k)]
    pw_full = np.asarray(inputs["projW"], dtype=np.float32)
    pb_full = np.asarray(inputs["projb"], dtype=np.float32)

    import ml_dtypes
    bfl = ml_dtypes.bfloat16
    shared = {
        "wq": np.ascontiguousarray(np.asarray(inputs["Wq"], dtype=bfl)),
        "wk": np.ascontiguousarray(np.asarray(inputs["Wk"], dtype=bfl)),
        "wv": np.ascontiguousarray(np.asarray(inputs["Wv"], dtype=bfl)),
        "wo": np.ascontiguousarray(np.asarray(inputs["Wo"], dtype=bfl)),
        "w1": np.ascontiguousarray(np.asarray(inputs["W1"], dtype=bfl)),
        "w2": np.ascontiguousarray(np.asarray(inputs["W2"], dtype=bfl)),
        "b1": np.ascontiguousarray(inputs["b1"], dtype=np.float32),
    }
    if not (ln1_triv and ln2_triv):
        lngb = np.stack([
            np.broadcast_to(ln1_g[:, None, :], (L, 128, D)),
            np.broadcast_to(ln1_b[:, None, :], (L, 128, D)),
            np.broadcast_to(ln2_g[:, None, :], (L, 128, D)),
            np.broadcast_to(ln2_b[:, None, :], (L, 128, D)),
        ], axis=1)
        shared["lngb"] = np.ascontiguousarray(lngb, dtype=np.float32)
    if not b2_triv:
        shared["b2b"] = np.ascontiguousarray(
            np.broadcast_to(b2[:, None, :], (L, 128, D)), dtype=np.float32)
    if not lnf_triv:
        shared["fgb"] = np.ascontiguousarray(
            np.stack([np.broadcast_to(lnf_g[None, :], (128, D)),
                      np.broadcast_to(lnf_b[None, :], (128, D))]), dtype=np.float32)

    in_maps = []
    for c in range(NC):
        b = c // 4
        q0 = (c % 4) * TOK
        sl = slice(b * S + q0, b * S + q0 + TOK)
        embx = np.ascontiguousarray(emb[ids[sl]], dtype=np.float32)
        pes = np.ascontiguousarray(pe_full[q0:q0 + TOK], dtype=np.float32)
        # mask tiles in scores-T layout: mt[kc, k, q] = mask[q0+q, kc*128+k]
        msl = m2d[q0:q0 + TOK, :]  # [TOK, S]
        mt = np.ascontiguousarray(
            np.asarray(msl.T.reshape(NKC, 128, TOK), dtype=bfl))
        pw = np.zeros((D, VPAD), dtype=bfl)
        pw[:, :VSH] = np.asarray(pw_full[:, c * VSH:(c + 1) * VSH], dtype=bfl)
        pb = np.zeros((VPAD,), dtype=np.float32)
        pb[:VSH] = pb_full[c * VSH:(c + 1) * VSH]
        im = dict(shared)
        im.update({"embx": embx, "pe": pes, "mt": mt,
                   "pw": pw, "pb": np.ascontiguousarray(pb)})
        in_maps.append(im)

    trace = bool(int(os.environ.get("KERNEL_TRACE", "0")))
    if trace:
        _install_trace_hook()
    res = run_bass_kernel_spmd(nc, in_maps, core_ids=list(range(NC)), trace=trace)
    if trace:
        kernel.last_exec_time_ns = res.exec_time_ns

    parts = [res.results[c]["logits"][:VSH, :] for c in range(NC)]
    full = np.concatenate(parts, axis=0)          # [V, B*S]
    return np.ascontiguousarray(full.T.reshape(B, S, V))


def _install_trace_hook():
    import types
    if 'antenv.axon_hooks' in sys.modules:
        return
    try:
        import trn_agent_boot.trn_boot as trn_boot
        mod = types.ModuleType('antenv.axon_hooks')
        _hook = [None]
        mod.set_axon_ntff_profile_hook = lambda hk: _hook.__setitem__(0, hk)
        mod.get_axon_ntff_profile_hook = lambda: _hook[0]
        sys.modules['antenv.axon_hooks'] = mod
        import antenv
        antenv.axon_hooks = mod
        mod.set_axon_ntff_profile_hook(
            trn_boot._ntff_profile_via_ctypes('/opt/axon/libaxon_pjrt.so'))
    except Exception as e:
        print(f"trace hook unavailable: {e}", file=sys.stderr)



# revision 6
# speedup vs baseline: 1.2846x; 1.2846x over previous
import sys, os
sys.path.insert(0, '/opt/trn_rl_repo')
import numpy as np
import concourse.bass as bass
import concourse.tile as tile
from concourse import bacc, mybir
from concourse.bass_utils import run_bass_kernel_spmd
from concourse.masks import make_identity

dt = mybir.dt
f32, f32r, bf16 = dt.float32, dt.float32r, dt.bfloat16

B, S, D, DFF, H, V, L = 2, 1024, 1024, 4096, 16, 32000, 8
DH = D // H          # 64
NC = 8               # cores
TOK = (B * S) // NC  # 256 tokens per core
NTB = TOK // 128     # 2 token blocks per core
NDC = D // 128       # 8 d-chunks
NKC = S // 128       # 8 key chunks per batch
NFFC = DFF // 128    # 32
VSH = V // NC        # 4000 vocab per core
VPAD = 4096          # padded vocab slice
EPS = 1e-5


def _pe():
    pos = np.arange(S, dtype=np.float32)[:, None]
    div = np.exp(np.arange(0, D, 2, dtype=np.float32) * (-np.log(10000.0) / D))
    pe = np.zeros((S, D), dtype=np.float32)
    pe[:, 0::2] = np.sin(pos * div)
    pe[:, 1::2] = np.cos(pos * div)
    return pe


def build_program(ln1_triv, ln2_triv, b2_triv, lnf_triv):
    nc = bacc.Bacc("TRN2", target_bir_lowering=False, debug=False,
                   enable_asserts=False, num_devices=NC)

    # ---- DRAM inputs (shared weights) ----
    wq_d = nc.dram_tensor("wq", [L, D, D], bf16, kind="ExternalInput").ap()
    wk_d = nc.dram_tensor("wk", [L, D, D], bf16, kind="ExternalInput").ap()
    wv_d = nc.dram_tensor("wv", [L, D, D], bf16, kind="ExternalInput").ap()
    wo_d = nc.dram_tensor("wo", [L, D, D], bf16, kind="ExternalInput").ap()
    w1_d = nc.dram_tensor("w1", [L, D, DFF], bf16, kind="ExternalInput").ap()
    w2_d = nc.dram_tensor("w2", [L, DFF, D], bf16, kind="ExternalInput").ap()
    b1_d = nc.dram_tensor("b1", [L, DFF], f32, kind="ExternalInput").ap()
    # ---- per-core inputs ----
    embx_d = nc.dram_tensor("embx", [TOK, D], f32, kind="ExternalInput").ap()
    pe_d = nc.dram_tensor("pe", [TOK, D], f32, kind="ExternalInput").ap()
    # causal mask for diagonal 128x128 blocks, [k, q] layout (triu)
    cm_d = nc.dram_tensor("cmask", [128, 128], bf16, kind="ExternalInput").ap()
    pw_d = nc.dram_tensor("pw", [D, VPAD], bf16, kind="ExternalInput").ap()
    pb_d = nc.dram_tensor("pb", [VPAD], f32, kind="ExternalInput").ap()
    if not (ln1_triv and ln2_triv):
        lngb_d = nc.dram_tensor("lngb", [L, 4, 128, D], f32, kind="ExternalInput").ap()
    if not b2_triv:
        b2b_d = nc.dram_tensor("b2b", [L, 128, D], f32, kind="ExternalInput").ap()
    if not lnf_triv:
        fgb_d = nc.dram_tensor("fgb", [2, 128, D], f32, kind="ExternalInput").ap()
    # ---- output ----
    out_d = nc.dram_tensor("logits", [VPAD, B * S], f32, kind="ExternalOutput").ap()

    from contextlib import ExitStack
    with tile.TileContext(nc) as tc:
        with ExitStack() as ctx:
            cpool = ctx.enter_context(tc.tile_pool(name="const", bufs=1))
            rpool = ctx.enter_context(tc.tile_pool(name="resid", bufs=1))
            alnpool = ctx.enter_context(tc.tile_pool(name="aln", bufs=2))
            atpool = ctx.enter_context(tc.tile_pool(name="aT", bufs=2))
            kqpool = ctx.enter_context(tc.tile_pool(name="kqv", bufs=1))
            gpool = ctx.enter_context(tc.tile_pool(name="gath", bufs=1))
            a2pool = ctx.enter_context(tc.tile_pool(name="at2", bufs=2))
            wpool = ctx.enter_context(tc.tile_pool(name="wch", bufs=3))
            w2pool = ctx.enter_context(tc.tile_pool(name="w2c", bufs=3))
            upool = ctx.enter_context(tc.tile_pool(name="ut", bufs=1))
            htpool = ctx.enter_context(tc.tile_pool(name="ht", bufs=2))
            lpool = ctx.enter_context(tc.tile_pool(name="lsb", bufs=2))
            spool = ctx.enter_context(tc.tile_pool(name="small", bufs=4))
            recpool = ctx.enter_context(tc.tile_pool(name="recp", bufs=2))
            psmm = ctx.enter_context(tc.tile_pool(name="ps_mm", bufs=6, space="PSUM"))
            pso = ctx.enter_context(tc.tile_pool(name="ps_o", bufs=2, space="PSUM"))
            dpool = ctx.enter_context(tc.tile_pool(name="dram", bufs=1, space="DRAM"))

            # ---------------- constants ----------------
            ident = cpool.tile([128, 128], bf16, tag="ident")
            make_identity(nc, ident[:])
            ones = cpool.tile([128, 64], bf16, tag="ones")
            nc.vector.memset(ones[:], 1.0)
            epst = cpool.tile([128, 1], f32, tag="eps")
            nc.vector.memset(epst[:], EPS)
            cmt = cpool.tile([128, 128], bf16, tag="cm")
            nc.sync.dma_start(out=cmt[:], in_=cm_d)
            b1t = cpool.tile([128, L, NFFC], f32, tag="b1")
            nc.sync.dma_start(out=b1t[:], in_=b1_d.rearrange("l (fc p) -> p l fc", p=128))
            pbt = cpool.tile([128, VPAD // 128], f32, tag="pb")
            nc.sync.dma_start(out=pbt[:], in_=pb_d.rearrange("(vc p) -> p vc", p=128))
            if not (ln1_triv and ln2_triv):
                lngb = cpool.tile([128, L, 4, D], f32, tag="lngb")
                nc.sync.dma_start(out=lngb[:], in_=lngb_d.rearrange("l f p d -> p l f d"))
            if not b2_triv:
                b2b = cpool.tile([128, L, D], f32, tag="b2b")
                nc.sync.dma_start(out=b2b[:], in_=b2b_d.rearrange("l p d -> p l d"))
            if not lnf_triv:
                fgb = cpool.tile([128, 2, D], f32, tag="fgb")
                nc.sync.dma_start(out=fgb[:], in_=fgb_d.rearrange("f p d -> p f d"))

            # collective DRAM buffers (flat; chunk j = dest/src rank j)
            kq_in = dpool.tile([NC * 2 * 128 * TOK], bf16, tag="kq_in")
            kq_out = dpool.tile([NC * 2 * 128 * TOK], bf16, tag="kq_out")
            v_in = dpool.tile([NC * NTB * 128 * 128], bf16, tag="v_in")
            v_out = dpool.tile([NC * NTB * 128 * 128], bf16, tag="v_out")
            o_in = dpool.tile([NC * 128 * TOK], bf16, tag="o_in")
            o_out = dpool.tile([NC * 128 * TOK], bf16, tag="o_out")
            ag2_in = dpool.tile([TOK * D], bf16, tag="ag2_in")
            ag2_out = dpool.tile([NC, TOK * D], bf16, tag="ag2_out", addr_space="Shared")

            # persistent SBUF tiles (reused across layers)
            h = [rpool.tile([128, D], f32, tag=f"h{tb}", name=f"h{tb}") for tb in range(NTB)]
            kT = kqpool.tile([128, NDC, TOK], bf16, tag="kT")
            qT = kqpool.tile([128, NDC, TOK], bf16, tag="qT")
            vloc = [kqpool.tile([128, D], bf16, tag=f"vloc{tb}", name=f"vloc{tb}")
                    for tb in range(NTB)]
            Kg = [gpool.tile([128, S], bf16, tag=f"Kg{b}", name=f"Kg{b}") for b in range(B)]
            Qg = [gpool.tile([128, S], bf16, tag=f"Qg{b}", name=f"Qg{b}") for b in range(B)]
            vt = [gpool.tile([128, NKC, 130], bf16, tag=f"vt{b}", name=f"vt{b}")
                  for b in range(B)]
            oTl = [gpool.tile([128, S], bf16, tag=f"oTl{b}", name=f"oTl{b}") for b in range(B)]
            oTfull = gpool.tile([128, NDC, TOK], bf16, tag="oTfull")
            # ones columns of vt (once; data DMAs each layer only touch cols 0:64/65:129)
            for b in range(B):
                nc.vector.memset(vt[b][:, :, 64:65], 1.0)
                nc.vector.memset(vt[b][:, :, 129:130], 1.0)

            # ---------------- embedding ----------------
            for tb in range(NTB):
                et = lpool.tile([128, D], f32, tag="lsb")
                pt = lpool.tile([128, D], f32, tag="lsb")
                nc.sync.dma_start(out=et, in_=embx_d[tb * 128:(tb + 1) * 128, :])
                nc.sync.dma_start(out=pt, in_=pe_d[tb * 128:(tb + 1) * 128, :])
                nc.vector.tensor_scalar(out=et[:], in0=et[:], scalar1=float(np.sqrt(D)),
                                        scalar2=None, op0=mybir.AluOpType.mult)
                nc.vector.tensor_add(h[tb][:], et[:], pt[:])

            def layer_norm(src_tiles, gb=None):
                outs = []
                for tb in range(NTB):
                    st = spool.tile([128, 2, 6], f32, tag="bnst")
                    xin = src_tiles[tb][:].rearrange("p (g d) -> p g d", g=2)
                    for g in range(2):
                        nc.vector.bn_stats(out=st[:, g, :], in_=xin[:, g, :])
                    mv = spool.tile([128, 2], f32, tag="bnmv")
                    nc.vector.bn_aggr(out=mv[:], in_=st[:])
                    std = spool.tile([128, 1], f32, tag="bnsd")
                    nc.scalar.activation(out=std[:], in_=mv[:, 1:2],
                                         func=mybir.ActivationFunctionType.Sqrt,
                                         bias=epst[:], scale=1.0)
                    nc.vector.reciprocal(out=std[:], in_=std[:])
                    at = alnpool.tile([128, D], bf16, tag="aln")
                    nc.vector.tensor_scalar(out=at[:], in0=src_tiles[tb][:],
                                            scalar1=mv[:, 0:1], scalar2=std[:],
                                            op0=mybir.AluOpType.subtract,
                                            op1=mybir.AluOpType.mult)
                    if gb is not None:
                        gt, bt = gb
                        nc.vector.tensor_mul(at[:], at[:], gt)
                        nc.vector.tensor_add(at[:], at[:], bt)
                    outs.append(at)
                return outs

            def transpose_to_aT(src_tiles):
                at = atpool.tile([128, NDC, TOK], bf16, tag="aT")
                for tb in range(NTB):
                    for dc in range(NDC):
                        ps = psmm.tile([128, 128], bf16, tag="mm")
                        nc.tensor.transpose(ps[:], src_tiles[tb][:, dc * 128:(dc + 1) * 128],
                                            ident[:])
                        nc.vector.tensor_copy(out=at[:, dc, tb * 128:(tb + 1) * 128], in_=ps[:])
                return at

            def proj_jc(wsrc_l, aT, dst):
                """dst[128, jc, TOK] = (W.T @ a.T); W [D, D] d-major chunks."""
                wsrc = wsrc_l.rearrange("(dc p) j -> p dc j", p=128)
                for cg in range(2):
                    wc = wpool.tile([128, NDC, 512], bf16, tag="wch")
                    nc.sync.dma_start(out=wc, in_=wsrc[:, :, cg * 512:(cg + 1) * 512])
                    for oc in range(4):
                        jc = cg * 4 + oc
                        ps = psmm.tile([128, TOK], f32, tag="mm")
                        for dc in range(NDC):
                            nc.tensor.matmul(ps[:], wc[:, dc, oc * 128:(oc + 1) * 128],
                                             aT[:, dc, :], start=(dc == 0), stop=(dc == NDC - 1))
                        nc.vector.tensor_copy(out=dst[:, jc, :], in_=ps[:])

            # views of the collective buffers
            kq_in_v = kq_in[:].rearrange("(j t p k) -> p j t k", j=NC, t=2, p=128)
            kq_out_v = kq_out[:].rearrange("(s t p k) -> p t s k", s=NC, t=2, p=128)
            v_in_v = v_in[:].rearrange("(j tb p e) -> p j tb e", j=NC, tb=NTB, p=128)
            v_out_v = v_out[:].rearrange("(s tb p e) -> p s tb e", s=NC, tb=NTB, p=128)
            o_in_v = o_in[:].rearrange("(j p k) -> p j k", j=NC, p=128)
            o_out_v = o_out[:].rearrange("(s p k) -> p s k", s=NC, p=128)

            # ---------------- layers ----------------
            for li in range(L):
                ln1_gb = None
                if not ln1_triv:
                    ln1_gb = (lngb[:, li, 0, :], lngb[:, li, 1, :])
                aln = layer_norm(h, ln1_gb)
                aT = transpose_to_aT(aln)

                # K, Q projections -> A2A
                proj_jc(wk_d[li], aT, kT)
                proj_jc(wq_d[li], aT, qT)
                nc.sync.dma_start(out=kq_in_v[:, :, 0, :], in_=kT[:])
                nc.sync.dma_start(out=kq_in_v[:, :, 1, :], in_=qT[:])
                nc.gpsimd.collective_compute(
                    "AllToAll", mybir.AluOpType.bypass,
                    replica_groups=[list(range(NC))],
                    ins=[kq_in[:].opt()], outs=[kq_out[:].opt()])

                # V projection (token-major) overlaps the KQ A2A
                wsrc = wv_d[li].rearrange("(dc p) j -> p dc j", p=128)
                for cg in range(2):
                    wc = wpool.tile([128, NDC, 512], bf16, tag="wch")
                    nc.sync.dma_start(out=wc, in_=wsrc[:, :, cg * 512:(cg + 1) * 512])
                    for tb in range(NTB):
                        ps = psmm.tile([128, 512], f32, tag="mm")
                        for dc in range(NDC):
                            nc.tensor.matmul(ps[:], aT[:, dc, tb * 128:(tb + 1) * 128],
                                             wc[:, dc, :], start=(dc == 0), stop=(dc == NDC - 1))
                        nc.vector.tensor_copy(out=vloc[tb][:, cg * 512:(cg + 1) * 512], in_=ps[:])
                for tb in range(NTB):
                    nc.sync.dma_start(
                        out=v_in_v[:, :, tb, :],
                        in_=vloc[tb][:].rearrange("p (j e) -> p j e", j=NC))
                nc.gpsimd.collective_compute(
                    "AllToAll", mybir.AluOpType.bypass,
                    replica_groups=[list(range(NC))],
                    ins=[v_in[:].opt()], outs=[v_out[:].opt()])

                # unpack gathered K/Q/V per batch
                for b in range(B):
                    nc.sync.dma_start(
                        out=Kg[b][:].rearrange("p (s k) -> p s k", s=4),
                        in_=kq_out_v[:, 0, 4 * b:4 * b + 4, :])
                    nc.sync.dma_start(
                        out=Qg[b][:].rearrange("p (s k) -> p s k", s=4),
                        in_=kq_out_v[:, 1, 4 * b:4 * b + 4, :])
                    vtv = vt[b][:].rearrange("p (g tb) c -> p g tb c", g=4)
                    nc.sync.dma_start(out=vtv[:, :, :, 0:64],
                                      in_=v_out_v[:, 4 * b:4 * b + 4, :, 0:64])
                    nc.sync.dma_start(out=vtv[:, :, :, 65:129],
                                      in_=v_out_v[:, 4 * b:4 * b + 4, :, 64:128])

                # ---- attention: 2 heads x 2 batches, causal ----
                for b in range(B):
                    at2 = [a2pool.tile([128, NKC, S], bf16, tag="at2", name=f"at2_{li}_{b}_{hh}")
                           for hh in range(2)]
                    for hh in range(2):
                        for kc in (1, 2, 3):
                            nc.vector.memset(at2[hh][:, kc, 0:kc * 128], 0.0)
                        for kc in (5, 6, 7):
                            nc.vector.memset(at2[hh][:, kc, 512:kc * 128], 0.0)
                    for kc in range(NKC):
                        q0 = kc * 128
                        spans = [(q0, 512), (512, S)] if q0 < 512 else [(q0, S)]
                        for (qa, qb) in spans:
                            n = qb - qa
                            for hh in range(2):
                                hb = hh * 64
                                ps = psmm.tile([128, 512], f32, tag="mm")
                                nc.tensor.matmul(
                                    ps[:, 0:n],
                                    Kg[b][hb:hb + 64, q0:q0 + 128],
                                    Qg[b][hb:hb + 64, qa:qb], start=True, stop=True)
                                nc.scalar.activation(
                                    out=at2[hh][:, kc, qa:qb], in_=ps[:, 0:n],
                                    func=mybir.ActivationFunctionType.Exp,
                                    scale=float(1.0 / np.sqrt(DH)))
                        for hh in range(2):
                            nc.vector.tensor_mul(at2[hh][:, kc, q0:q0 + 128],
                                                 at2[hh][:, kc, q0:q0 + 128], cmt[:])
                    for hh in range(2):
                        hb = hh * 64
                        for qc in range(2):
                            kcs = range(0, 4) if qc == 0 else range(0, 8)
                            ps_o = pso.tile([65, 512], f32, tag="o")
                            last = max(kcs)
                            for kc in kcs:
                                nc.tensor.matmul(
                                    ps_o[:], vt[b][:, kc, hh * 65:hh * 65 + 65],
                                    at2[hh][:, kc, qc * 512:(qc + 1) * 512],
                                    start=(kc == 0), stop=(kc == last))
                            rcp = recpool.tile([1, 512], f32, tag="rcp")
                            nc.vector.reciprocal(out=rcp[:], in_=ps_o[64:65, :])
                            rcpb = recpool.tile([1, 512], bf16, tag="rcpb")
                            with nc.allow_low_precision(reason="softmax denom"):
                                nc.vector.tensor_copy(out=rcpb[:], in_=rcp[:])
                            ps_rb = psmm.tile([64, 512], f32, tag="mm")
                            nc.tensor.matmul(ps_rb[:], ones[0:1, 0:64], rcpb[:],
                                             start=True, stop=True)
                            rbs = recpool.tile([64, 512], bf16, tag="rbs")
                            with nc.allow_low_precision(reason="softmax denom bcast"):
                                nc.vector.tensor_copy(out=rbs[:], in_=ps_rb[:])
                            nc.vector.tensor_mul(oTl[b][hb:hb + 64, qc * 512:(qc + 1) * 512],
                                                 ps_o[0:64, :], rbs[:])

                # ---- A2A o back to token-sharded, then Wo + residual ----
                for b in range(B):
                    nc.sync.dma_start(
                        out=o_in_v[:, 4 * b:4 * b + 4, :],
                        in_=oTl[b][:].rearrange("p (s k) -> p s k", s=4))
                nc.gpsimd.collective_compute(
                    "AllToAll", mybir.AluOpType.bypass,
                    replica_groups=[list(range(NC))],
                    ins=[o_in[:].opt()], outs=[o_out[:].opt()])
                nc.sync.dma_start(out=oTfull[:], in_=o_out_v)

                wsrc = wo_d[li].rearrange("(jc p) o -> p jc o", p=128)
                for cg in range(2):
                    wc = wpool.tile([128, NDC, 512], bf16, tag="wch")
                    nc.sync.dma_start(out=wc, in_=wsrc[:, :, cg * 512:(cg + 1) * 512])
                    for tb in range(NTB):
                        ps = psmm.tile([128, 512], f32, tag="mm")
                        for jc in range(NDC):
                            nc.tensor.matmul(ps[:], oTfull[:, jc, tb * 128:(tb + 1) * 128],
                                             wc[:, jc, :], start=(jc == 0), stop=(jc == NDC - 1))
                        nc.vector.tensor_add(h[tb][:, cg * 512:(cg + 1) * 512],
                                             h[tb][:, cg * 512:(cg + 1) * 512], ps[:])

                # ---- FFN ----
                ln2_gb = None
                if not ln2_triv:
                    ln2_gb = (lngb[:, li, 2, :], lngb[:, li, 3, :])
                f_ln = layer_norm(h, ln2_gb)
                fT = transpose_to_aT(f_ln)

                uts = upool.tile([128, NFFC, TOK], bf16, tag="ut")
                wsrc = w1_d[li].rearrange("(dc p) j -> p dc j", p=128)
                for cg in range(NFFC // 4):
                    wc = wpool.tile([128, NDC, 512], bf16, tag="wch")
                    nc.sync.dma_start(out=wc, in_=wsrc[:, :, cg * 512:(cg + 1) * 512])
                    for oc in range(4):
                        fc = cg * 4 + oc
                        ps = psmm.tile([128, TOK], f32, tag="mm")
                        for dc in range(NDC):
                            nc.tensor.matmul(ps[:], wc[:, dc, oc * 128:(oc + 1) * 128],
                                             fT[:, dc, :], start=(dc == 0), stop=(dc == NDC - 1))
                        nc.vector.tensor_scalar(out=uts[:, fc, :], in0=ps[:],
                                                scalar1=b1t[:, li, fc:fc + 1], scalar2=0.0,
                                                op0=mybir.AluOpType.add,
                                                op1=mybir.AluOpType.max)

                wsrc = w2_d[li].rearrange("(fc p) o -> p fc o", p=128)
                chains = {}
                for tb in range(NTB):
                    for og in range(2):
                        chains[(tb, og)] = psmm.tile([128, 512], f32, tag="mm",
                                                     name=f"w2ch{li}{tb}{og}")
                for fcg in range(NFFC // 2):
                    wc = w2pool.tile([128, 2, D], bf16, tag="w2c")
                    nc.sync.dma_start(out=wc, in_=wsrc[:, fcg * 2:fcg * 2 + 2, :])
                    for f2 in range(2):
                        fc = fcg * 2 + f2
                        for tb in range(NTB):
                            for og in range(2):
                                nc.tensor.matmul(chains[(tb, og)][:],
                                                 uts[:, fc, tb * 128:(tb + 1) * 128],
                                                 wc[:, f2, og * 512:(og + 1) * 512],
                                                 start=(fc == 0), stop=(fc == NFFC - 1))
                for tb in range(NTB):
                    for og in range(2):
                        nc.vector.tensor_add(h[tb][:, og * 512:(og + 1) * 512],
                                             h[tb][:, og * 512:(og + 1) * 512],
                                             chains[(tb, og)][:])
                    if not b2_triv:
                        nc.vector.tensor_add(h[tb][:], h[tb][:], b2b[:, li, :])

            # ---------------- final LN + AllGather + projection ----------------
            fin_gb = None if lnf_triv else (fgb[:, 0, :], fgb[:, 1, :])
            fin = layer_norm(h, fin_gb)
            finT = transpose_to_aT(fin)
            nc.sync.dma_start(
                out=ag2_in[:].rearrange("(jc p t) -> p jc t", p=128, t=TOK),
                in_=finT[:])
            nc.gpsimd.collective_compute(
                "AllGather", mybir.AluOpType.bypass,
                replica_groups=[list(range(NC))],
                ins=[ag2_in[:].opt()], outs=[ag2_out[:].opt()])

            hfull = ag2_out[:].rearrange("s (jc p t) -> p s jc t", jc=NDC, p=128, t=TOK)
            htl = []
            for half in range(2):
                ht = htpool.tile([128, NDC, 1024], bf16, tag="ht", name=f"ht{half}")
                for s in range(4):
                    nc.sync.dma_start(
                        out=ht[:, :, s * TOK:(s + 1) * TOK],
                        in_=hfull[:, 4 * half + s, :, :])
                htl.append(ht)

            pwsrc = pw_d.rearrange("(dc p) v -> p dc v", p=128)
            for wcg in range(VPAD // 512):
                pwc = wpool.tile([128, NDC, 512], bf16, tag="wch")
                nc.sync.dma_start(out=pwc, in_=pwsrc[:, :, wcg * 512:(wcg + 1) * 512])
                for vc in range(4):
                    vg = wcg * 4 + vc
                    for half in range(2):
                        lsb = lpool.tile([128, 1024], f32, tag="lsb")
                        for tq in range(2):
                            ps = psmm.tile([128, 512], f32, tag="mm")
                            for dc in range(NDC):
                                nc.tensor.matmul(ps[:], pwc[:, dc, vc * 128:(vc + 1) * 128],
                                                 htl[half][:, dc, tq * 512:(tq + 1) * 512],
                                                 start=(dc == 0), stop=(dc == NDC - 1))
                            nc.vector.tensor_scalar(out=lsb[:, tq * 512:(tq + 1) * 512],
                                                    in0=ps[:],
                                                    scalar1=pbt[:, vg:vg + 1], scalar2=None,
                                                    op0=mybir.AluOpType.add)
                        nc.sync.dma_start(
                            out=out_d[vg * 128:(vg + 1) * 128,
                                      half * 1024:(half + 1) * 1024],
                            in_=lsb[:])
    nc.compile()
    return nc


def kernel(**inputs):
    x = np.asarray(inputs["x"])
    emb = np.asarray(inputs["emb"], dtype=np.float32)

    ln1_g = np.asarray(inputs["ln1_g"], dtype=np.float32)
    ln1_b = np.asarray(inputs["ln1_b"], dtype=np.float32)
    ln2_g = np.asarray(inputs["ln2_g"], dtype=np.float32)
    ln2_b = np.asarray(inputs["ln2_b"], dtype=np.float32)
    lnf_g = np.asarray(inputs["lnf_g"], dtype=np.float32)
    lnf_b = np.asarray(inputs["lnf_b"], dtype=np.float32)
    b2 = np.asarray(inputs["b2"], dtype=np.float32)

    ln1_triv = bool(np.all(ln1_g == 1) and np.all(ln1_b == 0))
    ln2_triv = bool(np.all(ln2_g == 1) and np.all(ln2_b == 0))
    lnf_triv = bool(np.all(lnf_g == 1) and np.all(lnf_b == 0))
    b2_triv = bool(np.all(b2 == 0))

    nc = build_program(ln1_triv, ln2_triv, b2_triv, lnf_triv)

    pe_full = _pe()
    ids = np.asarray(x).reshape(B * S)
    pw_full = np.asarray(inputs["projW"], dtype=np.float32)
    pb_full = np.asarray(inputs["projb"], dtype=np.float32)

    import ml_dtypes
    bfl = ml_dtypes.bfloat16
    # causal mask in [k, q] layout for the diagonal block: valid iff k <= q
    cmask = np.ascontiguousarray(
        np.triu(np.ones((128, 128), dtype=np.float32)).astype(bfl))
    shared = {
        "wq": np.ascontiguousarray(np.asarray(inputs["Wq"], dtype=bfl)),
        "wk": np.ascontiguousarray(np.asarray(inputs["Wk"], dtype=bfl)),
        "wv": np.ascontiguousarray(np.asarray(inputs["Wv"], dtype=bfl)),
        "wo": np.ascontiguousarray(np.asarray(inputs["Wo"], dtype=bfl)),
        "w1": np.ascontiguousarray(np.asarray(inputs["W1"], dtype=bfl)),
        "w2": np.ascontiguousarray(np.asarray(inputs["W2"], dtype=bfl)),
        "b1": np.ascontiguousarray(inputs["b1"], dtype=np.float32),
        "cmask": cmask,
    }
    if not (ln1_triv and ln2_triv):
        lngb = np.stack([
            np.broadcast_to(ln1_g[:, None, :], (L, 128, D)),
            np.broadcast_to(ln1_b[:, None, :], (L, 128, D)),
            np.broadcast_to(ln2_g[:, None, :], (L, 128, D)),
            np.broadcast_to(ln2_b[:, None, :], (L, 128, D)),
        ], axis=1)
        shared["lngb"] = np.ascontiguousarray(lngb, dtype=np.float32)
    if not b2_triv:
        shared["b2b"] = np.ascontiguousarray(
            np.broadcast_to(b2[:, None, :], (L, 128, D)), dtype=np.float32)
    if not lnf_triv:
        shared["fgb"] = np.ascontiguousarray(
            np.stack([np.broadcast_to(lnf_g[None, :], (128, D)),
                      np.broadcast_to(lnf_b[None, :], (128, D))]), dtype=np.float32)

    in_maps = []
    for c in range(NC):
        b = c // 4
        q0 = (c % 4) * TOK
        sl = slice(b * S + q0, b * S + q0 + TOK)
        embx = np.ascontiguousarray(emb[ids[sl]], dtype=np.float32)
        pes = np.ascontiguousarray(pe_full[q0:q0 + TOK], dtype=np.float32)
        pw = np.zeros((D, VPAD), dtype=bfl)
        pw[:, :VSH] = np.asarray(pw_full[:, c * VSH:(c + 1) * VSH], dtype=bfl)
        pb = np.zeros((VPAD,), dtype=np.float32)
        pb[:VSH] = pb_full[c * VSH:(c + 1) * VSH]
        im = dict(shared)
        im.update({"embx": embx, "pe": pes, "pw": pw, "pb": np.ascontiguousarray(pb)})
        in_maps.append(im)

    trace = bool(int(os.environ.get("KERNEL_TRACE", "0")))
    if trace:
        _install_trace_hook()
    res = run_bass_kernel_spmd(nc, in_maps, core_ids=list(range(NC)), trace=trace)
    if trace:
        kernel.last_exec_time_ns = res.exec_time_ns

    parts = [res.results[c]["logits"][:VSH, :] for c in range(NC)]
    full = np.concatenate(parts, axis=0)          # [V, B*S]
    return np.ascontiguousarray(full.T.reshape(B, S, V))


def _install_trace_hook():
    import types
    if 'antenv.axon_hooks' in sys.modules:
        return
    try:
        import trn_agent_boot.trn_boot as trn_boot
        mod = types.ModuleType('antenv.axon_hooks')
        _hook = [None]
        mod.set_axon_ntff_profile_hook = lambda hk: _hook.__setitem__(0, hk)
        mod.get_axon_ntff_profile_hook = lambda: _hook[0]
        sys.modules['antenv.axon_hooks'] = mod
        import antenv
        antenv.axon_hooks = mod
        mod.set_axon_ntff_profile_hook(
            trn_boot._ntff_profile_via_ctypes('/opt/axon/libaxon_pjrt.so'))
    except Exception as e:
        print(f"trace hook unavailable: {e}", file=sys.stderr)


# revision 11
# speedup vs baseline: 1.3072x; 1.0176x over previous
import sys, os
sys.path.insert(0, '/opt/trn_rl_repo')
import numpy as np
import concourse.bass as bass
import concourse.tile as tile
from concourse import bacc, mybir
from concourse.bass_utils import run_bass_kernel_spmd
from concourse.masks import make_identity

dt = mybir.dt
f32, f32r, bf16 = dt.float32, dt.float32r, dt.bfloat16

B, S, D, DFF, H, V, L = 2, 1024, 1024, 4096, 16, 32000, 8
DH = D // H          # 64
NC = 8               # cores
TOK = (B * S) // NC  # 256 tokens per core
NTB = TOK // 128     # 2 token blocks per core
NDC = D // 128       # 8 d-chunks
NKC = S // 128       # 8 key chunks per batch
NFFC = DFF // 128    # 32
VSH = V // NC        # 4000 vocab per core
VPAD = 4096          # padded vocab slice
EPS = 1e-5


def _pe():
    pos = np.arange(S, dtype=np.float32)[:, None]
    div = np.exp(np.arange(0, D, 2, dtype=np.float32) * (-np.log(10000.0) / D))
    pe = np.zeros((S, D), dtype=np.float32)
    pe[:, 0::2] = np.sin(pos * div)
    pe[:, 1::2] = np.cos(pos * div)
    return pe


def build_program(ln1_triv, ln2_triv, b2_triv, lnf_triv):
    nc = bacc.Bacc("TRN2", target_bir_lowering=False, debug=False,
                   enable_asserts=False, num_devices=NC)

    # ---- DRAM inputs (shared weights) ----
    wq_d = nc.dram_tensor("wq", [L, D, D], bf16, kind="ExternalInput").ap()
    wk_d = nc.dram_tensor("wk", [L, D, D], bf16, kind="ExternalInput").ap()
    wv_d = nc.dram_tensor("wv", [L, D, D], bf16, kind="ExternalInput").ap()
    wo_d = nc.dram_tensor("wo", [L, D, D], bf16, kind="ExternalInput").ap()
    w1_d = nc.dram_tensor("w1", [L, D, DFF], bf16, kind="ExternalInput").ap()
    w2_d = nc.dram_tensor("w2", [L, DFF, D], bf16, kind="ExternalInput").ap()
    b1_d = nc.dram_tensor("b1", [L, DFF], f32, kind="ExternalInput").ap()
    # ---- per-core inputs ----
    embx_d = nc.dram_tensor("embx", [TOK, D], f32, kind="ExternalInput").ap()
    pe_d = nc.dram_tensor("pe", [TOK, D], f32, kind="ExternalInput").ap()
    # causal mask for diagonal 128x128 blocks, [k, q] layout (triu)
    cm_d = nc.dram_tensor("cmask", [128, 128], bf16, kind="ExternalInput").ap()
    pw_d = nc.dram_tensor("pw", [D, VPAD], bf16, kind="ExternalInput").ap()
    pb_d = nc.dram_tensor("pb", [VPAD], f32, kind="ExternalInput").ap()
    if not (ln1_triv and ln2_triv):
        lngb_d = nc.dram_tensor("lngb", [L, 4, 128, D], f32, kind="ExternalInput").ap()
    if not b2_triv:
        b2b_d = nc.dram_tensor("b2b", [L, 128, D], f32, kind="ExternalInput").ap()
    if not lnf_triv:
        fgb_d = nc.dram_tensor("fgb", [2, 128, D], f32, kind="ExternalInput").ap()
    # ---- output ----
    out_d = nc.dram_tensor("logits", [VPAD, B * S], f32, kind="ExternalOutput").ap()

    from contextlib import ExitStack
    with tile.TileContext(nc) as tc:
        with ExitStack() as ctx:
            cpool = ctx.enter_context(tc.tile_pool(name="const", bufs=1))
            rpool = ctx.enter_context(tc.tile_pool(name="resid", bufs=1))
            alnpool = ctx.enter_context(tc.tile_pool(name="aln", bufs=2))
            atpool = ctx.enter_context(tc.tile_pool(name="aT", bufs=2))
            kqpool = ctx.enter_context(tc.tile_pool(name="kqv", bufs=1))
            gpool = ctx.enter_context(tc.tile_pool(name="gath", bufs=1))
            a2pool = ctx.enter_context(tc.tile_pool(name="at2", bufs=2))
            wpool = ctx.enter_context(tc.tile_pool(name="wch", bufs=3))
            w2pool = ctx.enter_context(tc.tile_pool(name="w2c", bufs=3))
            upool = ctx.enter_context(tc.tile_pool(name="ut", bufs=1))
            htpool = ctx.enter_context(tc.tile_pool(name="ht", bufs=2))
            lpool = ctx.enter_context(tc.tile_pool(name="lsb", bufs=2))
            spool = ctx.enter_context(tc.tile_pool(name="small", bufs=4))
            recpool = ctx.enter_context(tc.tile_pool(name="recp", bufs=2))
            psmm = ctx.enter_context(tc.tile_pool(name="ps_mm", bufs=6, space="PSUM"))
            pso = ctx.enter_context(tc.tile_pool(name="ps_o", bufs=2, space="PSUM"))
            dpool = ctx.enter_context(tc.tile_pool(name="dram", bufs=1, space="DRAM"))

            # ---------------- constants ----------------
            ident = cpool.tile([128, 128], bf16, tag="ident")
            make_identity(nc, ident[:])
            ones = cpool.tile([128, 64], bf16, tag="ones")
            nc.vector.memset(ones[:], 1.0)
            epst = cpool.tile([128, 1], f32, tag="eps")
            nc.vector.memset(epst[:], EPS)
            cmt = cpool.tile([128, 128], bf16, tag="cm")
            nc.sync.dma_start(out=cmt[:], in_=cm_d)
            b1t = cpool.tile([128, L, NFFC], f32, tag="b1")
            nc.sync.dma_start(out=b1t[:], in_=b1_d.rearrange("l (fc p) -> p l fc", p=128))
            pbt = cpool.tile([128, VPAD // 128], f32, tag="pb")
            nc.sync.dma_start(out=pbt[:], in_=pb_d.rearrange("(vc p) -> p vc", p=128))
            if not (ln1_triv and ln2_triv):
                lngb = cpool.tile([128, L, 4, D], f32, tag="lngb")
                nc.sync.dma_start(out=lngb[:], in_=lngb_d.rearrange("l f p d -> p l f d"))
            if not b2_triv:
                b2b = cpool.tile([128, L, D], f32, tag="b2b")
                nc.sync.dma_start(out=b2b[:], in_=b2b_d.rearrange("l p d -> p l d"))
            if not lnf_triv:
                fgb = cpool.tile([128, 2, D], f32, tag="fgb")
                nc.sync.dma_start(out=fgb[:], in_=fgb_d.rearrange("f p d -> p f d"))

            # collective DRAM buffers (flat; chunk j = dest/src rank j)
            kq_in = dpool.tile([NC * 2 * 128 * TOK], bf16, tag="kq_in")
            kq_out = dpool.tile([NC * 2 * 128 * TOK], bf16, tag="kq_out")
            v_in = dpool.tile([NC * NTB * 128 * 128], bf16, tag="v_in")
            v_out = dpool.tile([NC * NTB * 128 * 128], bf16, tag="v_out")
            o_in = dpool.tile([NC * 128 * TOK], bf16, tag="o_in")
            o_out = dpool.tile([NC * 128 * TOK], bf16, tag="o_out")
            ag2_in = dpool.tile([TOK * D], bf16, tag="ag2_in")
            ag2_out = dpool.tile([NC, TOK * D], bf16, tag="ag2_out", addr_space="Shared")
            wu_in = dpool.tile([NC * 128], bf16, tag="wu_in")
            wu_out = dpool.tile([NC * 128], bf16, tag="wu_out")

            # tiny warmup collective: absorbs comm-stack init + start skew
            nc.gpsimd.collective_compute(
                "AllToAll", mybir.AluOpType.bypass,
                replica_groups=[list(range(NC))],
                ins=[wu_in[:].opt()], outs=[wu_out[:].opt()])

            # persistent SBUF tiles (reused across layers)
            h = [rpool.tile([128, D], f32, tag=f"h{tb}", name=f"h{tb}") for tb in range(NTB)]
            kT = kqpool.tile([128, NDC, TOK], bf16, tag="kT")
            qT = kqpool.tile([128, NDC, TOK], bf16, tag="qT")
            vloc = [kqpool.tile([128, D], bf16, tag=f"vloc{tb}", name=f"vloc{tb}")
                    for tb in range(NTB)]
            Kg = [gpool.tile([128, S], bf16, tag=f"Kg{b}", name=f"Kg{b}") for b in range(B)]
            Qg = [gpool.tile([128, S], bf16, tag=f"Qg{b}", name=f"Qg{b}") for b in range(B)]
            vt = [gpool.tile([128, NKC, 130], bf16, tag=f"vt{b}", name=f"vt{b}")
                  for b in range(B)]
            oTl = [gpool.tile([128, S], bf16, tag=f"oTl{b}", name=f"oTl{b}") for b in range(B)]
            oTfull = gpool.tile([128, NDC, TOK], bf16, tag="oTfull")
            # ones columns of vt (once; data DMAs each layer only touch cols 0:64/65:129)
            for b in range(B):
                nc.vector.memset(vt[b][:, :, 64:65], 1.0)
                nc.vector.memset(vt[b][:, :, 129:130], 1.0)

            # ---------------- embedding ----------------
            for tb in range(NTB):
                et = lpool.tile([128, D], f32, tag="lsb")
                pt = lpool.tile([128, D], f32, tag="lsb")
                nc.sync.dma_start(out=et, in_=embx_d[tb * 128:(tb + 1) * 128, :])
                nc.sync.dma_start(out=pt, in_=pe_d[tb * 128:(tb + 1) * 128, :])
                nc.vector.tensor_scalar(out=et[:], in0=et[:], scalar1=float(np.sqrt(D)),
                                        scalar2=None, op0=mybir.AluOpType.mult)
                nc.vector.tensor_add(h[tb][:], et[:], pt[:])

            def layer_norm(src_tiles, gb=None):
                outs = []
                for tb in range(NTB):
                    st = spool.tile([128, 2, 6], f32, tag="bnst")
                    xin = src_tiles[tb][:].rearrange("p (g d) -> p g d", g=2)
                    for g in range(2):
                        nc.vector.bn_stats(out=st[:, g, :], in_=xin[:, g, :])
                    mv = spool.tile([128, 2], f32, tag="bnmv")
                    nc.vector.bn_aggr(out=mv[:], in_=st[:])
                    std = spool.tile([128, 1], f32, tag="bnsd")
                    nc.scalar.activation(out=std[:], in_=mv[:, 1:2],
                                         func=mybir.ActivationFunctionType.Sqrt,
                                         bias=epst[:], scale=1.0)
                    nc.vector.reciprocal(out=std[:], in_=std[:])
                    at = alnpool.tile([128, D], bf16, tag="aln")
                    nc.vector.tensor_scalar(out=at[:], in0=src_tiles[tb][:],
                                            scalar1=mv[:, 0:1], scalar2=std[:],
                                            op0=mybir.AluOpType.subtract,
                                            op1=mybir.AluOpType.mult)
                    if gb is not None:
                        gt, bt = gb
                        nc.vector.tensor_mul(at[:], at[:], gt)
                        nc.vector.tensor_add(at[:], at[:], bt)
                    outs.append(at)
                return outs

            def transpose_to_aT(src_tiles):
                at = atpool.tile([128, NDC, TOK], bf16, tag="aT")
                for tb in range(NTB):
                    for dc in range(NDC):
                        ps = psmm.tile([128, 128], bf16, tag="mm")
                        nc.tensor.transpose(ps[:], src_tiles[tb][:, dc * 128:(dc + 1) * 128],
                                            ident[:])
                        nc.vector.tensor_copy(out=at[:, dc, tb * 128:(tb + 1) * 128], in_=ps[:])
                return at

            def proj_jc(wsrc_l, aT, dst):
                """dst[128, jc, TOK] = (W.T @ a.T); W [D, D] d-major chunks."""
                wsrc = wsrc_l.rearrange("(dc p) j -> p dc j", p=128)
                for cg in range(2):
                    wc = wpool.tile([128, NDC, 512], bf16, tag="wch")
                    nc.sync.dma_start(out=wc, in_=wsrc[:, :, cg * 512:(cg + 1) * 512])
                    for oc in range(4):
                        jc = cg * 4 + oc
                        ps = psmm.tile([128, TOK], f32, tag="mm")
                        for dc in range(NDC):
                            nc.tensor.matmul(ps[:], wc[:, dc, oc * 128:(oc + 1) * 128],
                                             aT[:, dc, :], start=(dc == 0), stop=(dc == NDC - 1))
                        nc.vector.tensor_copy(out=dst[:, jc, :], in_=ps[:])

            # views of the collective buffers
            kq_in_v = kq_in[:].rearrange("(j t p k) -> p j t k", j=NC, t=2, p=128)
            kq_out_v = kq_out[:].rearrange("(s t p k) -> p t s k", s=NC, t=2, p=128)
            v_in_v = v_in[:].rearrange("(j tb p e) -> p j tb e", j=NC, tb=NTB, p=128)
            v_out_v = v_out[:].rearrange("(s tb p e) -> p s tb e", s=NC, tb=NTB, p=128)
            o_in_v = o_in[:].rearrange("(j p k) -> p j k", j=NC, p=128)
            o_out_v = o_out[:].rearrange("(s p k) -> p s k", s=NC, p=128)

            # ---------------- layers ----------------
            for li in range(L):
                ln1_gb = None
                if not ln1_triv:
                    ln1_gb = (lngb[:, li, 0, :], lngb[:, li, 1, :])
                aln = layer_norm(h, ln1_gb)
                aT = transpose_to_aT(aln)

                # K, Q projections -> A2A
                proj_jc(wk_d[li], aT, kT)
                proj_jc(wq_d[li], aT, qT)
                nc.sync.dma_start(out=kq_in_v[:, :, 0, :], in_=kT[:])
                nc.sync.dma_start(out=kq_in_v[:, :, 1, :], in_=qT[:])
                nc.gpsimd.collective_compute(
                    "AllToAll", mybir.AluOpType.bypass,
                    replica_groups=[list(range(NC))],
                    ins=[kq_in[:].opt()], outs=[kq_out[:].opt()])

                # V projection (token-major) overlaps the KQ A2A
                wsrc = wv_d[li].rearrange("(dc p) j -> p dc j", p=128)
                for cg in range(2):
                    wc = wpool.tile([128, NDC, 512], bf16, tag="wch")
                    nc.sync.dma_start(out=wc, in_=wsrc[:, :, cg * 512:(cg + 1) * 512])
                    for tb in range(NTB):
                        ps = psmm.tile([128, 512], f32, tag="mm")
                        for dc in range(NDC):
                            nc.tensor.matmul(ps[:], aT[:, dc, tb * 128:(tb + 1) * 128],
                                             wc[:, dc, :], start=(dc == 0), stop=(dc == NDC - 1))
                        nc.vector.tensor_copy(out=vloc[tb][:, cg * 512:(cg + 1) * 512], in_=ps[:])
                for tb in range(NTB):
                    nc.sync.dma_start(
                        out=v_in_v[:, :, tb, :],
                        in_=vloc[tb][:].rearrange("p (j e) -> p j e", j=NC))
                nc.gpsimd.collective_compute(
                    "AllToAll", mybir.AluOpType.bypass,
                    replica_groups=[list(range(NC))],
                    ins=[v_in[:].opt()], outs=[v_out[:].opt()])

                # unpack gathered K/Q/V per batch
                for b in range(B):
                    nc.scalar.dma_start(
                        out=Kg[b][:].rearrange("p (s k) -> p s k", s=4),
                        in_=kq_out_v[:, 0, 4 * b:4 * b + 4, :])
                    nc.scalar.dma_start(
                        out=Qg[b][:].rearrange("p (s k) -> p s k", s=4),
                        in_=kq_out_v[:, 1, 4 * b:4 * b + 4, :])
                    vtv = vt[b][:].rearrange("p (g tb) c -> p g tb c", g=4)
                    nc.scalar.dma_start(out=vtv[:, :, :, 0:64],
                                      in_=v_out_v[:, 4 * b:4 * b + 4, :, 0:64])
                    nc.scalar.dma_start(out=vtv[:, :, :, 65:129],
                                      in_=v_out_v[:, 4 * b:4 * b + 4, :, 64:128])

                # ---- attention: 2 heads x 2 batches, causal ----
                for b in range(B):
                    at2 = [a2pool.tile([128, NKC, S], bf16, tag="at2", name=f"at2_{li}_{b}_{hh}")
                           for hh in range(2)]
                    for hh in range(2):
                        for kc in (1, 2, 3):
                            nc.vector.memset(at2[hh][:, kc, 0:kc * 128], 0.0)
                        for kc in (5, 6, 7):
                            nc.vector.memset(at2[hh][:, kc, 512:kc * 128], 0.0)
                    for kc in range(NKC):
                        q0 = kc * 128
                        spans = [(q0, 512), (512, S)] if q0 < 512 else [(q0, S)]
                        for (qa, qb) in spans:
                            n = qb - qa
                            for hh in range(2):
                                hb = hh * 64
                                ps = psmm.tile([128, 512], f32, tag="mm")
                                nc.tensor.matmul(
                                    ps[:, 0:n],
                                    Kg[b][hb:hb + 64, q0:q0 + 128],
                                    Qg[b][hb:hb + 64, qa:qb], start=True, stop=True)
                                nc.scalar.activation(
                                    out=at2[hh][:, kc, qa:qb], in_=ps[:, 0:n],
                                    func=mybir.ActivationFunctionType.Exp,
                                    scale=float(1.0 / np.sqrt(DH)))
                        for hh in range(2):
                            nc.vector.tensor_mul(at2[hh][:, kc, q0:q0 + 128],
                                                 at2[hh][:, kc, q0:q0 + 128], cmt[:])
                    for hh in range(2):
                        hb = hh * 64
                        for qc in range(2):
                            kcs = range(0, 4) if qc == 0 else range(0, 8)
                            ps_o = pso.tile([65, 512], f32, tag="o")
                            last = max(kcs)
                            for kc in kcs:
                                nc.tensor.matmul(
                                    ps_o[:], vt[b][:, kc, hh * 65:hh * 65 + 65],
                                    at2[hh][:, kc, qc * 512:(qc + 1) * 512],
                                    start=(kc == 0), stop=(kc == last))
                            ssb = recpool.tile([1, 512], bf16, tag="ssb")
                            with nc.allow_low_precision(reason="softmax denom"):
                                nc.vector.tensor_copy(out=ssb[:], in_=ps_o[64:65, :])
                            rcpb = recpool.tile([1, 512], bf16, tag="rcpb")
                            with nc.allow_low_precision(reason="softmax denom recip"):
                                nc.vector.reciprocal(out=rcpb[:], in_=ssb[:])
                            ps_rb = psmm.tile([64, 512], f32, tag="mm")
                            nc.tensor.matmul(ps_rb[:], ones[0:1, 0:64], rcpb[:],
                                             start=True, stop=True)
                            rbs = recpool.tile([64, 512], bf16, tag="rbs")
                            with nc.allow_low_precision(reason="softmax denom bcast"):
                                nc.vector.tensor_copy(out=rbs[:], in_=ps_rb[:])
                            nc.vector.tensor_mul(oTl[b][hb:hb + 64, qc * 512:(qc + 1) * 512],
                                                 ps_o[0:64, :], rbs[:])

                # ---- A2A o back to token-sharded, then Wo + residual ----
                for b in range(B):
                    nc.sync.dma_start(
                        out=o_in_v[:, 4 * b:4 * b + 4, :],
                        in_=oTl[b][:].rearrange("p (s k) -> p s k", s=4))
                nc.gpsimd.collective_compute(
                    "AllToAll", mybir.AluOpType.bypass,
                    replica_groups=[list(range(NC))],
                    ins=[o_in[:].opt()], outs=[o_out[:].opt()])
                nc.scalar.dma_start(out=oTfull[:], in_=o_out_v)

                wsrc = wo_d[li].rearrange("(jc p) o -> p jc o", p=128)
                for cg in range(2):
                    wc = wpool.tile([128, NDC, 512], bf16, tag="wch")
                    nc.sync.dma_start(out=wc, in_=wsrc[:, :, cg * 512:(cg + 1) * 512])
                    for tb in range(NTB):
                        ps = psmm.tile([128, 512], f32, tag="mm")
                        for jc in range(NDC):
                            nc.tensor.matmul(ps[:], oTfull[:, jc, tb * 128:(tb + 1) * 128],
                                             wc[:, jc, :], start=(jc == 0), stop=(jc == NDC - 1))
                        nc.vector.tensor_add(h[tb][:, cg * 512:(cg + 1) * 512],
                                             h[tb][:, cg * 512:(cg + 1) * 512], ps[:])

                # ---- FFN ----
                ln2_gb = None
                if not ln2_triv:
                    ln2_gb = (lngb[:, li, 2, :], lngb[:, li, 3, :])
                f_ln = layer_norm(h, ln2_gb)
                fT = transpose_to_aT(f_ln)

                uts = upool.tile([128, NFFC, TOK], bf16, tag="ut")
                wsrc = w1_d[li].rearrange("(dc p) j -> p dc j", p=128)
                for cg in range(NFFC // 4):
                    wc = wpool.tile([128, NDC, 512], bf16, tag="wch")
                    nc.sync.dma_start(out=wc, in_=wsrc[:, :, cg * 512:(cg + 1) * 512])
                    for oc in range(4):
                        fc = cg * 4 + oc
                        ps = psmm.tile([128, TOK], f32, tag="mm")
                        for dc in range(NDC):
                            nc.tensor.matmul(ps[:], wc[:, dc, oc * 128:(oc + 1) * 128],
                                             fT[:, dc, :], start=(dc == 0), stop=(dc == NDC - 1))
                        nc.scalar.activation(out=uts[:, fc, :], in_=ps[:],
                                             func=mybir.ActivationFunctionType.Relu,
                                             bias=b1t[:, li, fc:fc + 1], scale=1.0)

                wsrc = w2_d[li].rearrange("(fc p) o -> p fc o", p=128)
                chains = {}
                for tb in range(NTB):
                    for og in range(2):
                        chains[(tb, og)] = psmm.tile([128, 512], f32, tag="mm",
                                                     name=f"w2ch{li}{tb}{og}")
                for fcg in range(NFFC // 2):
                    wc = w2pool.tile([128, 2, D], bf16, tag="w2c")
                    nc.sync.dma_start(out=wc, in_=wsrc[:, fcg * 2:fcg * 2 + 2, :])
                    for f2 in range(2):
                        fc = fcg * 2 + f2
                        for tb in range(NTB):
                            for og in range(2):
                                nc.tensor.matmul(chains[(tb, og)][:],
                                                 uts[:, fc, tb * 128:(tb + 1) * 128],
                                                 wc[:, f2, og * 512:(og + 1) * 512],
                                                 start=(fc == 0), stop=(fc == NFFC - 1))
                for tb in range(NTB):
                    for og in range(2):
                        nc.vector.tensor_add(h[tb][:, og * 512:(og + 1) * 512],
                                             h[tb][:, og * 512:(og + 1) * 512],
                                             chains[(tb, og)][:])
                    if not b2_triv:
                        nc.vector.tensor_add(h[tb][:], h[tb][:], b2b[:, li, :])

            # ---------------- final LN + AllGather + projection ----------------
            fin_gb = None if lnf_triv else (fgb[:, 0, :], fgb[:, 1, :])
            fin = layer_norm(h, fin_gb)
            finT = transpose_to_aT(fin)
            nc.sync.dma_start(
                out=ag2_in[:].rearrange("(jc p t) -> p jc t", p=128, t=TOK),
                in_=finT[:])
            nc.gpsimd.collective_compute(
                "AllGather", mybir.AluOpType.bypass,
                replica_groups=[list(range(NC))],
                ins=[ag2_in[:].opt()], outs=[ag2_out[:].opt()])

            hfull = ag2_out[:].rearrange("s (jc p t) -> p s jc t", jc=NDC, p=128, t=TOK)
            htl = []
            for half in range(2):
                ht = htpool.tile([128, NDC, 1024], bf16, tag="ht", name=f"ht{half}")
                for s in range(4):
                    nc.scalar.dma_start(
                        out=ht[:, :, s * TOK:(s + 1) * TOK],
                        in_=hfull[:, 4 * half + s, :, :])
                htl.append(ht)

            pwsrc = pw_d.rearrange("(dc p) v -> p dc v", p=128)
            for wcg in range(VPAD // 512):
                pwc = wpool.tile([128, NDC, 512], bf16, tag="wch")
                nc.sync.dma_start(out=pwc, in_=pwsrc[:, :, wcg * 512:(wcg + 1) * 512])
                for vc in range(4):
                    vg = wcg * 4 + vc
                    for half in range(2):
                        lsb = lpool.tile([128, 1024], f32, tag="lsb")
                        for tq in range(2):
                            ps = psmm.tile([128, 512], f32, tag="mm")
                            for dc in range(NDC):
                                nc.tensor.matmul(ps[:], pwc[:, dc, vc * 128:(vc + 1) * 128],
                                                 htl[half][:, dc, tq * 512:(tq + 1) * 512],
                                                 start=(dc == 0), stop=(dc == NDC - 1))
                            nc.vector.tensor_scalar(out=lsb[:, tq * 512:(tq + 1) * 512],
                                                    in0=ps[:],
                                                    scalar1=pbt[:, vg:vg + 1], scalar2=None,
                                                    op0=mybir.AluOpType.add)
                        nc.sync.dma_start(
                            out=out_d[vg * 128:(vg + 1) * 128,
                                      half * 1024:(half + 1) * 1024],
                            in_=lsb[:])
    nc.compile()
    return nc


def kernel(**inputs):
    x = np.asarray(inputs["x"])
    emb = np.asarray(inputs["emb"], dtype=np.float32)

    ln1_g = np.asarray(inputs["ln1_g"], dtype=np.float32)
    ln1_b = np.asarray(inputs["ln1_b"], dtype=np.float32)
    ln2_g = np.asarray(inputs["ln2_g"], dtype=np.float32)
    ln2_b = np.asarray(inputs["ln2_b"], dtype=np.float32)
    lnf_g = np.asarray(inputs["lnf_g"], dtype=np.float32)
    lnf_b = np.asarray(inputs["lnf_b"], dtype=np.float32)
    b2 = np.asarray(inputs["b2"], dtype=np.float32)

    ln1_triv = bool(np.all(ln1_g == 1) and np.all(ln1_b == 0))
    ln2_triv = bool(np.all(ln2_g == 1) and np.all(ln2_b == 0))
    lnf_triv = bool(np.all(lnf_g == 1) and np.all(lnf_b == 0))
    b2_triv = bool(np.all(b2 == 0))

    nc = build_program(ln1_triv, ln2_triv, b2_triv, lnf_triv)

    pe_full = _pe()
    ids = np.asarray(x).reshape(B * S)
    pw_full = np.asarray(inputs["projW"], dtype=np.float32)
    pb_full = np.asarray(inputs["projb"], dtype=np.float32)

    import ml_dtypes
    bfl = ml_dtypes.bfloat16
    # causal mask in [k, q] layout for the diagonal block: valid iff k <= q
    cmask = np.ascontiguousarray(
        np.triu(np.ones((128, 128), dtype=np.float32)).astype(bfl))
    shared = {
        "wq": np.ascontiguousarray(np.asarray(inputs["Wq"], dtype=bfl)),
        "wk": np.ascontiguousarray(np.asarray(inputs["Wk"], dtype=bfl)),
        "wv": np.ascontiguousarray(np.asarray(inputs["Wv"], dtype=bfl)),
        "wo": np.ascontiguousarray(np.asarray(inputs["Wo"], dtype=bfl)),
        "w1": np.ascontiguousarray(np.asarray(inputs["W1"], dtype=bfl)),
        "w2": np.ascontiguousarray(np.asarray(inputs["W2"], dtype=bfl)),
        "b1": np.ascontiguousarray(inputs["b1"], dtype=np.float32),
        "cmask": cmask,
    }
    if not (ln1_triv and ln2_triv):
        lngb = np.stack([
            np.broadcast_to(ln1_g[:, None, :], (L, 128, D)),
            np.broadcast_to(ln1_b[:, None, :], (L, 128, D)),
            np.broadcast_to(ln2_g[:, None, :], (L, 128, D)),
            np.broadcast_to(ln2_b[:, None, :], (L, 128, D)),
        ], axis=1)
        shared["lngb"] = np.ascontiguousarray(lngb, dtype=np.float32)
    if not b2_triv:
        shared["b2b"] = np.ascontiguousarray(
            np.broadcast_to(b2[:, None, :], (L, 128, D)), dtype=np.float32)
    if not lnf_triv:
        shared["fgb"] = np.ascontiguousarray(
            np.stack([np.broadcast_to(lnf_g[None, :], (128, D)),
                      np.broadcast_to(lnf_b[None, :], (128, D))]), dtype=np.float32)

    in_maps = []
    for c in range(NC):
        b = c // 4
        q0 = (c % 4) * TOK
        sl = slice(b * S + q0, b * S + q0 + TOK)
        embx = np.ascontiguousarray(emb[ids[sl]], dtype=np.float32)
        pes = np.ascontiguousarray(pe_full[q0:q0 + TOK], dtype=np.float32)
        pw = np.zeros((D, VPAD), dtype=bfl)
        pw[:, :VSH] = np.asarray(pw_full[:, c * VSH:(c + 1) * VSH], dtype=bfl)
        pb = np.zeros((VPAD,), dtype=np.float32)
        pb[:VSH] = pb_full[c * VSH:(c + 1) * VSH]
        im = dict(shared)
        im.update({"embx": embx, "pe": pes, "pw": pw, "pb": np.ascontiguousarray(pb)})
        in_maps.append(im)

    trace = bool(int(os.environ.get("KERNEL_TRACE", "0")))
    if trace:
        _install_trace_hook()
    res = run_bass_kernel_spmd(nc, in_maps, core_ids=list(range(NC)), trace=trace)
    if trace:
        kernel.last_exec_time_ns = res.exec_time_ns

    parts = [res.results[c]["logits"][:VSH, :] for c in range(NC)]
    full = np.concatenate(parts, axis=0)          # [V, B*S]
    return np.ascontiguousarray(full.T.reshape(B, S, V))


def _install_trace_hook():
    import types
    if 'antenv.axon_hooks' in sys.modules:
        return
    try:
        import trn_agent_boot.trn_boot as trn_boot
        mod = types.ModuleType('antenv.axon_hooks')
        _hook = [None]
        mod.set_axon_ntff_profile_hook = lambda hk: _hook.__setitem__(0, hk)
        mod.get_axon_ntff_profile_hook = lambda: _hook[0]
        sys.modules['antenv.axon_hooks'] = mod
        import antenv
        antenv.axon_hooks = mod
        mod.set_axon_ntff_profile_hook(
            trn_boot._ntff_profile_via_ctypes('/opt/axon/libaxon_pjrt.so'))
    except Exception as e:
        print(f"trace hook unavailable: {e}", file=sys.stderr)


# revision 13
# speedup vs baseline: 1.4415x; 1.1027x over previous
import sys, os
sys.path.insert(0, '/opt/trn_rl_repo')
import numpy as np
import concourse.bass as bass
import concourse.tile as tile
from concourse import bacc, mybir
from concourse.bass_utils import run_bass_kernel_spmd
from concourse.masks import make_identity

dt = mybir.dt
f32, f32r, bf16 = dt.float32, dt.float32r, dt.bfloat16

B, S, D, DFF, H, V, L = 2, 1024, 1024, 4096, 16, 32000, 8
DH = D // H          # 64
NC = 8               # cores
TOK = (B * S) // NC  # 256 tokens per core
NTB = TOK // 128     # 2 token blocks per core
NDC = D // 128       # 8 d-chunks
NKC = S // 128       # 8 key chunks per batch
NFFC = DFF // 128    # 32
VSH = V // NC        # 4000 vocab per core
VPAD = 4096          # padded vocab slice
EPS = 1e-5
ASCALE = 16.0        # fp8 scale for K/Q/V a2a payloads


def _pe():
    pos = np.arange(S, dtype=np.float32)[:, None]
    div = np.exp(np.arange(0, D, 2, dtype=np.float32) * (-np.log(10000.0) / D))
    pe = np.zeros((S, D), dtype=np.float32)
    pe[:, 0::2] = np.sin(pos * div)
    pe[:, 1::2] = np.cos(pos * div)
    return pe


def build_program(ln1_triv, ln2_triv, b2_triv, lnf_triv):
    nc = bacc.Bacc("TRN2", target_bir_lowering=False, debug=False,
                   enable_asserts=False, num_devices=NC)

    # ---- DRAM inputs (shared weights) ----
    wq_d = nc.dram_tensor("wq", [L, D, D], bf16, kind="ExternalInput").ap()
    wk_d = nc.dram_tensor("wk", [L, D, D], bf16, kind="ExternalInput").ap()
    wv_d = nc.dram_tensor("wv", [L, D, D], bf16, kind="ExternalInput").ap()
    wo_d = nc.dram_tensor("wo", [L, D, D], bf16, kind="ExternalInput").ap()
    f8 = dt.float8e4
    w1_d = nc.dram_tensor("w1", [L, D, DFF], bf16, kind="ExternalInput").ap()
    w2_d = nc.dram_tensor("w2", [L, DFF, D], bf16, kind="ExternalInput").ap()
    b1_d = nc.dram_tensor("b1", [L, DFF], f32, kind="ExternalInput").ap()
    # ---- per-core inputs ----
    embx_d = nc.dram_tensor("embx", [TOK, D], f32, kind="ExternalInput").ap()
    pe_d = nc.dram_tensor("pe", [TOK, D], f32, kind="ExternalInput").ap()
    # causal mask for diagonal 128x128 blocks, [k, q] layout (triu)
    cm_d = nc.dram_tensor("cmask", [128, 128], bf16, kind="ExternalInput").ap()
    pw_d = nc.dram_tensor("pw", [D, VPAD], bf16, kind="ExternalInput").ap()
    pb_d = nc.dram_tensor("pb", [VPAD], f32, kind="ExternalInput").ap()
    if not (ln1_triv and ln2_triv):
        lngb_d = nc.dram_tensor("lngb", [L, 4, 128, D], f32, kind="ExternalInput").ap()
    if not b2_triv:
        b2b_d = nc.dram_tensor("b2b", [L, 128, D], f32, kind="ExternalInput").ap()
    if not lnf_triv:
        fgb_d = nc.dram_tensor("fgb", [2, 128, D], f32, kind="ExternalInput").ap()
    # ---- output ----
    out_d = nc.dram_tensor("logits", [VPAD, B * S], f32, kind="ExternalOutput").ap()

    from contextlib import ExitStack
    with tile.TileContext(nc) as tc:
        with ExitStack() as ctx:
            cpool = ctx.enter_context(tc.tile_pool(name="const", bufs=1))
            rpool = ctx.enter_context(tc.tile_pool(name="resid", bufs=1))
            alnpool = ctx.enter_context(tc.tile_pool(name="aln", bufs=2))
            atpool = ctx.enter_context(tc.tile_pool(name="aT", bufs=2))
            kqpool = ctx.enter_context(tc.tile_pool(name="kqv", bufs=1))
            gpool = ctx.enter_context(tc.tile_pool(name="gath", bufs=1))
            a2pool = ctx.enter_context(tc.tile_pool(name="at2", bufs=2))
            wpool = ctx.enter_context(tc.tile_pool(name="wch", bufs=3))
            w2pool = ctx.enter_context(tc.tile_pool(name="w2c", bufs=3))
            upool = ctx.enter_context(tc.tile_pool(name="ut", bufs=1))
            htpool = ctx.enter_context(tc.tile_pool(name="ht", bufs=2))
            lpool = ctx.enter_context(tc.tile_pool(name="lsb", bufs=2))
            spool = ctx.enter_context(tc.tile_pool(name="small", bufs=4))
            recpool = ctx.enter_context(tc.tile_pool(name="recp", bufs=2))
            psmm = ctx.enter_context(tc.tile_pool(name="ps_mm", bufs=6, space="PSUM"))
            pso = ctx.enter_context(tc.tile_pool(name="ps_o", bufs=2, space="PSUM"))
            dpool = ctx.enter_context(tc.tile_pool(name="dram", bufs=1, space="DRAM"))

            # ---------------- constants ----------------
            ident = cpool.tile([128, 128], bf16, tag="ident")
            make_identity(nc, ident[:])
            ones = cpool.tile([128, 64], bf16, tag="ones")
            nc.vector.memset(ones[:], 1.0)
            epst = cpool.tile([128, 1], f32, tag="eps")
            nc.vector.memset(epst[:], EPS)
            nlsc = cpool.tile([128, 1], f32, tag="nlsc")
            nc.vector.memset(nlsc[:], float(-np.log(ASCALE)))
            cmt = cpool.tile([128, 128], bf16, tag="cm")
            nc.sync.dma_start(out=cmt[:], in_=cm_d)
            b1t = cpool.tile([128, L, NFFC], f32, tag="b1")
            nc.sync.dma_start(out=b1t[:], in_=b1_d.rearrange("l (fc p) -> p l fc", p=128))
            pbt = cpool.tile([128, VPAD // 128], f32, tag="pb")
            nc.sync.dma_start(out=pbt[:], in_=pb_d.rearrange("(vc p) -> p vc", p=128))
            if not (ln1_triv and ln2_triv):
                lngb = cpool.tile([128, L, 4, D], f32, tag="lngb")
                nc.sync.dma_start(out=lngb[:], in_=lngb_d.rearrange("l f p d -> p l f d"))
            if not b2_triv:
                b2b = cpool.tile([128, L, D], f32, tag="b2b")
                nc.sync.dma_start(out=b2b[:], in_=b2b_d.rearrange("l p d -> p l d"))
            if not lnf_triv:
                fgb = cpool.tile([128, 2, D], f32, tag="fgb")
                nc.sync.dma_start(out=fgb[:], in_=fgb_d.rearrange("f p d -> p f d"))

            # collective DRAM buffers (flat; chunk j = dest/src rank j)
            kq_in = dpool.tile([NC * 2 * 128 * TOK], f8, tag="kq_in")
            kq_out = dpool.tile([NC * 2 * 128 * TOK], f8, tag="kq_out")
            v_in = dpool.tile([NC * NTB * 128 * 128], f8, tag="v_in")
            v_out = dpool.tile([NC * NTB * 128 * 128], f8, tag="v_out")
            o_in = dpool.tile([NC * 128 * TOK], bf16, tag="o_in")
            o_out = dpool.tile([NC * 128 * TOK], bf16, tag="o_out")
            ag2_in = dpool.tile([TOK * D], bf16, tag="ag2_in")
            ag2_out = dpool.tile([NC, TOK * D], bf16, tag="ag2_out", addr_space="Shared")
            wu_in = dpool.tile([NC * 128], bf16, tag="wu_in")
            wu_out = dpool.tile([NC * 128], bf16, tag="wu_out")

            # tiny warmup collective: absorbs comm-stack init + start skew
            nc.gpsimd.collective_compute(
                "AllToAll", mybir.AluOpType.bypass,
                replica_groups=[list(range(NC))],
                ins=[wu_in[:].opt()], outs=[wu_out[:].opt()])

            # persistent SBUF tiles (reused across layers)
            h = [rpool.tile([128, D], f32, tag=f"h{tb}", name=f"h{tb}") for tb in range(NTB)]
            kT = kqpool.tile([128, NDC, TOK], f8, tag="kT")
            qT = kqpool.tile([128, NDC, TOK], f8, tag="qT")
            vloc = [kqpool.tile([128, D], f8, tag=f"vloc{tb}", name=f"vloc{tb}")
                    for tb in range(NTB)]
            Kg = [gpool.tile([128, S], f8, tag=f"Kg{b}", name=f"Kg{b}") for b in range(B)]
            Qg = [gpool.tile([128, S], f8, tag=f"Qg{b}", name=f"Qg{b}") for b in range(B)]
            vt = [gpool.tile([128, NKC, 130], f8, tag=f"vt{b}", name=f"vt{b}")
                  for b in range(B)]
            oTl = [gpool.tile([128, S], bf16, tag=f"oTl{b}", name=f"oTl{b}") for b in range(B)]
            oTfull = gpool.tile([128, NDC, TOK], bf16, tag="oTfull")
            # ones columns of vt (once; data DMAs each layer only touch cols 0:64/65:129)
            for b in range(B):
                nc.vector.memset(vt[b][:, :, 64:65], 1.0)
                nc.vector.memset(vt[b][:, :, 129:130], 1.0)

            # ---------------- embedding ----------------
            for tb in range(NTB):
                et = lpool.tile([128, D], f32, tag="lsb")
                pt = lpool.tile([128, D], f32, tag="lsb")
                nc.sync.dma_start(out=et, in_=embx_d[tb * 128:(tb + 1) * 128, :])
                nc.sync.dma_start(out=pt, in_=pe_d[tb * 128:(tb + 1) * 128, :])
                nc.vector.tensor_scalar(out=et[:], in0=et[:], scalar1=float(np.sqrt(D)),
                                        scalar2=None, op0=mybir.AluOpType.mult)
                nc.vector.tensor_add(h[tb][:], et[:], pt[:])

            def layer_norm(src_tiles, gb=None):
                outs = []
                for tb in range(NTB):
                    st = spool.tile([128, 2, 6], f32, tag="bnst")
                    xin = src_tiles[tb][:].rearrange("p (g d) -> p g d", g=2)
                    for g in range(2):
                        nc.vector.bn_stats(out=st[:, g, :], in_=xin[:, g, :])
                    mv = spool.tile([128, 2], f32, tag="bnmv")
                    nc.vector.bn_aggr(out=mv[:], in_=st[:])
                    std = spool.tile([128, 1], f32, tag="bnsd")
                    nc.scalar.activation(out=std[:], in_=mv[:, 1:2],
                                         func=mybir.ActivationFunctionType.Sqrt,
                                         bias=epst[:], scale=1.0)
                    nc.vector.reciprocal(out=std[:], in_=std[:])
                    at = alnpool.tile([128, D], bf16, tag="aln")
                    nc.vector.tensor_scalar(out=at[:], in0=src_tiles[tb][:],
                                            scalar1=mv[:, 0:1], scalar2=std[:],
                                            op0=mybir.AluOpType.subtract,
                                            op1=mybir.AluOpType.mult)
                    if gb is not None:
                        gt, bt = gb
                        nc.vector.tensor_mul(at[:], at[:], gt)
                        nc.vector.tensor_add(at[:], at[:], bt)
                    outs.append(at)
                return outs

            def transpose_to_aT(src_tiles):
                at = atpool.tile([128, NDC, TOK], bf16, tag="aT")
                for tb in range(NTB):
                    for dc in range(NDC):
                        ps = psmm.tile([128, 128], bf16, tag="mm")
                        nc.tensor.transpose(ps[:], src_tiles[tb][:, dc * 128:(dc + 1) * 128],
                                            ident[:])
                        nc.vector.tensor_copy(out=at[:, dc, tb * 128:(tb + 1) * 128], in_=ps[:])
                return at

            def proj_jc(wsrc_l, aT, dst):
                """dst[128, jc, TOK] = (W.T @ a.T); W [D, D] d-major chunks."""
                wsrc = wsrc_l.rearrange("(dc p) j -> p dc j", p=128)
                for cg in range(2):
                    wc = wpool.tile([128, NDC, 512], bf16, tag="wch")
                    nc.sync.dma_start(out=wc, in_=wsrc[:, :, cg * 512:(cg + 1) * 512])
                    for oc in range(4):
                        jc = cg * 4 + oc
                        ps = psmm.tile([128, TOK], f32, tag="mm")
                        for dc in range(NDC):
                            nc.tensor.matmul(ps[:], wc[:, dc, oc * 128:(oc + 1) * 128],
                                             aT[:, dc, :], start=(dc == 0), stop=(dc == NDC - 1))
                        with nc.allow_low_precision(reason="fp8 a2a payload"):
                            nc.vector.tensor_scalar(out=dst[:, jc, :], in0=ps[:],
                                                    scalar1=ASCALE, scalar2=None,
                                                    op0=mybir.AluOpType.mult)

            # views of the collective buffers
            kq_in_v = kq_in[:].rearrange("(j t p k) -> p j t k", j=NC, t=2, p=128)
            kq_out_v = kq_out[:].rearrange("(s t p k) -> p t s k", s=NC, t=2, p=128)
            v_in_v = v_in[:].rearrange("(j tb p e) -> p j tb e", j=NC, tb=NTB, p=128)
            v_out_v = v_out[:].rearrange("(s tb p e) -> p s tb e", s=NC, tb=NTB, p=128)
            o_in_v = o_in[:].rearrange("(j p k) -> p j k", j=NC, p=128)
            o_out_v = o_out[:].rearrange("(s p k) -> p s k", s=NC, p=128)

            # ---------------- layers ----------------
            for li in range(L):
                ln1_gb = None
                if not ln1_triv:
                    ln1_gb = (lngb[:, li, 0, :], lngb[:, li, 1, :])
                aln = layer_norm(h, ln1_gb)
                aT = transpose_to_aT(aln)

                # K, Q projections -> A2A
                proj_jc(wk_d[li], aT, kT)
                proj_jc(wq_d[li], aT, qT)
                nc.sync.dma_start(out=kq_in_v[:, :, 0, :], in_=kT[:])
                nc.sync.dma_start(out=kq_in_v[:, :, 1, :], in_=qT[:])
                nc.gpsimd.collective_compute(
                    "AllToAll", mybir.AluOpType.bypass,
                    replica_groups=[list(range(NC))],
                    ins=[kq_in[:].opt()], outs=[kq_out[:].opt()])

                # V projection (token-major) overlaps the KQ A2A
                wsrc = wv_d[li].rearrange("(dc p) j -> p dc j", p=128)
                for cg in range(2):
                    wc = wpool.tile([128, NDC, 512], bf16, tag="wch")
                    nc.sync.dma_start(out=wc, in_=wsrc[:, :, cg * 512:(cg + 1) * 512])
                    for tb in range(NTB):
                        ps = psmm.tile([128, 512], f32, tag="mm")
                        for dc in range(NDC):
                            nc.tensor.matmul(ps[:], aT[:, dc, tb * 128:(tb + 1) * 128],
                                             wc[:, dc, :], start=(dc == 0), stop=(dc == NDC - 1))
                        with nc.allow_low_precision(reason="fp8 a2a payload"):
                            nc.vector.tensor_scalar(
                                out=vloc[tb][:, cg * 512:(cg + 1) * 512], in0=ps[:],
                                scalar1=ASCALE, scalar2=None,
                                op0=mybir.AluOpType.mult)
                for tb in range(NTB):
                    nc.sync.dma_start(
                        out=v_in_v[:, :, tb, :],
                        in_=vloc[tb][:].rearrange("p (j e) -> p j e", j=NC))
                nc.gpsimd.collective_compute(
                    "AllToAll", mybir.AluOpType.bypass,
                    replica_groups=[list(range(NC))],
                    ins=[v_in[:].opt()], outs=[v_out[:].opt()])

                # unpack gathered K/Q/V per batch
                for b in range(B):
                    nc.scalar.dma_start(
                        out=Kg[b][:].rearrange("p (s k) -> p s k", s=4),
                        in_=kq_out_v[:, 0, 4 * b:4 * b + 4, :])
                    nc.scalar.dma_start(
                        out=Qg[b][:].rearrange("p (s k) -> p s k", s=4),
                        in_=kq_out_v[:, 1, 4 * b:4 * b + 4, :])
                    vtv = vt[b][:].rearrange("p (g tb) c -> p g tb c", g=4)
                    nc.scalar.dma_start(out=vtv[:, :, :, 0:64],
                                      in_=v_out_v[:, 4 * b:4 * b + 4, :, 0:64])
                    nc.scalar.dma_start(out=vtv[:, :, :, 65:129],
                                      in_=v_out_v[:, 4 * b:4 * b + 4, :, 64:128])

                # ---- attention: 2 heads x 2 batches, causal ----
                for b in range(B):
                    at2 = [a2pool.tile([128, NKC, S], bf16, tag="at2", name=f"at2_{li}_{b}_{hh}")
                           for hh in range(2)]
                    for hh in range(2):
                        for kc in (1, 2, 3):
                            nc.vector.memset(at2[hh][:, kc, 0:kc * 128], 0.0)
                        for kc in (5, 6, 7):
                            nc.vector.memset(at2[hh][:, kc, 512:kc * 128], 0.0)
                    for kc in range(NKC):
                        q0 = kc * 128
                        spans = [(q0, 512), (512, S)] if q0 < 512 else [(q0, S)]
                        for (qa, qb) in spans:
                            n = qb - qa
                            for hh in range(2):
                                hb = hh * 64
                                ps = psmm.tile([128, 512], f32, tag="mm")
                                nc.tensor.matmul(
                                    ps[:, 0:n],
                                    Kg[b][hb:hb + 64, q0:q0 + 128],
                                    Qg[b][hb:hb + 64, qa:qb], start=True, stop=True)
                                nc.scalar.activation(
                                    out=at2[hh][:, kc, qa:qb], in_=ps[:, 0:n],
                                    func=mybir.ActivationFunctionType.Exp,
                                    scale=float(1.0 / (ASCALE * ASCALE * np.sqrt(DH))))
                        for hh in range(2):
                            nc.vector.tensor_mul(at2[hh][:, kc, q0:q0 + 128],
                                                 at2[hh][:, kc, q0:q0 + 128], cmt[:])
                    for hh in range(2):
                        hb = hh * 64
                        for qc in range(2):
                            kcs = range(0, 4) if qc == 0 else range(0, 8)
                            ps_o = pso.tile([65, 512], f32, tag="o")
                            last = max(kcs)
                            for kc in kcs:
                                nc.tensor.matmul(
                                    ps_o[:], vt[b][:, kc, hh * 65:hh * 65 + 65],
                                    at2[hh][:, kc, qc * 512:(qc + 1) * 512],
                                    start=(kc == 0), stop=(kc == last))
                            ssb = recpool.tile([1, 512], bf16, tag="ssb")
                            with nc.allow_low_precision(reason="softmax denom"):
                                nc.vector.tensor_copy(out=ssb[:], in_=ps_o[64:65, :])
                            ps_rb = psmm.tile([64, 512], f32, tag="mm")
                            nc.tensor.matmul(ps_rb[:], ones[0:1, 0:64], ssb[:],
                                             start=True, stop=True)
                            lnt = recpool.tile([64, 512], f32, tag="lnt")
                            nc.scalar.activation(out=lnt[:], in_=ps_rb[:],
                                                 func=mybir.ActivationFunctionType.Ln,
                                                 scale=1.0)
                            rbs = recpool.tile([64, 512], bf16, tag="rbs")
                            nc.scalar.activation(out=rbs[:], in_=lnt[:],
                                                 func=mybir.ActivationFunctionType.Exp,
                                                 scale=-1.0, bias=nlsc[0:64, :])
                            nc.vector.tensor_mul(oTl[b][hb:hb + 64, qc * 512:(qc + 1) * 512],
                                                 ps_o[0:64, :], rbs[:])

                # ---- A2A o back to token-sharded, then Wo + residual ----
                for b in range(B):
                    nc.sync.dma_start(
                        out=o_in_v[:, 4 * b:4 * b + 4, :],
                        in_=oTl[b][:].rearrange("p (s k) -> p s k", s=4))
                nc.gpsimd.collective_compute(
                    "AllToAll", mybir.AluOpType.bypass,
                    replica_groups=[list(range(NC))],
                    ins=[o_in[:].opt()], outs=[o_out[:].opt()])
                nc.scalar.dma_start(out=oTfull[:], in_=o_out_v)

                wsrc = wo_d[li].rearrange("(jc p) o -> p jc o", p=128)
                for cg in range(2):
                    wc = wpool.tile([128, NDC, 512], bf16, tag="wch")
                    nc.sync.dma_start(out=wc, in_=wsrc[:, :, cg * 512:(cg + 1) * 512])
                    for tb in range(NTB):
                        ps = psmm.tile([128, 512], f32, tag="mm")
                        for jc in range(NDC):
                            nc.tensor.matmul(ps[:], oTfull[:, jc, tb * 128:(tb + 1) * 128],
                                             wc[:, jc, :], start=(jc == 0), stop=(jc == NDC - 1))
                        nc.vector.tensor_add(h[tb][:, cg * 512:(cg + 1) * 512],
                                             h[tb][:, cg * 512:(cg + 1) * 512], ps[:])

                # ---- FFN ----
                ln2_gb = None
                if not ln2_triv:
                    ln2_gb = (lngb[:, li, 2, :], lngb[:, li, 3, :])
                f_ln = layer_norm(h, ln2_gb)
                fT = transpose_to_aT(f_ln)

                uts = upool.tile([128, NFFC, TOK], bf16, tag="ut")
                wsrc = w1_d[li].rearrange("(dc p) j -> p dc j", p=128)
                for cg in range(NFFC // 4):
                    wc = wpool.tile([128, NDC, 512], bf16, tag="wch")
                    nc.sync.dma_start(out=wc, in_=wsrc[:, :, cg * 512:(cg + 1) * 512])
                    for oc in range(4):
                        fc = cg * 4 + oc
                        ps = psmm.tile([128, TOK], f32, tag="mm")
                        for dc in range(NDC):
                            nc.tensor.matmul(ps[:], wc[:, dc, oc * 128:(oc + 1) * 128],
                                             fT[:, dc, :], start=(dc == 0), stop=(dc == NDC - 1))
                        nc.scalar.activation(out=uts[:, fc, :], in_=ps[:],
                                             func=mybir.ActivationFunctionType.Relu,
                                             bias=b1t[:, li, fc:fc + 1], scale=1.0)

                wsrc = w2_d[li].rearrange("(fc p) o -> p fc o", p=128)
                chains = {}
                for tb in range(NTB):
                    for og in range(2):
                        chains[(tb, og)] = psmm.tile([128, 512], f32, tag="mm",
                                                     name=f"w2ch{li}{tb}{og}")
                for fcg in range(NFFC // 2):
                    wc = w2pool.tile([128, 2, D], bf16, tag="w2c")
                    nc.sync.dma_start(out=wc, in_=wsrc[:, fcg * 2:fcg * 2 + 2, :])
                    for f2 in range(2):
                        fc = fcg * 2 + f2
                        for tb in range(NTB):
                            for og in range(2):
                                nc.tensor.matmul(chains[(tb, og)][:],
                                                 uts[:, fc, tb * 128:(tb + 1) * 128],
                                                 wc[:, f2, og * 512:(og + 1) * 512],
                                                 start=(fc == 0), stop=(fc == NFFC - 1))
                for tb in range(NTB):
                    for og in range(2):
                        nc.vector.tensor_add(h[tb][:, og * 512:(og + 1) * 512],
                                             h[tb][:, og * 512:(og + 1) * 512],
                                             chains[(tb, og)][:])
                    if not b2_triv:
                        nc.vector.tensor_add(h[tb][:], h[tb][:], b2b[:, li, :])

            # ---------------- final LN + AllGather + projection ----------------
            fin_gb = None if lnf_triv else (fgb[:, 0, :], fgb[:, 1, :])
            fin = layer_norm(h, fin_gb)
            finT = transpose_to_aT(fin)
            nc.sync.dma_start(
                out=ag2_in[:].rearrange("(jc p t) -> p jc t", p=128, t=TOK),
                in_=finT[:])
            nc.gpsimd.collective_compute(
                "AllGather", mybir.AluOpType.bypass,
                replica_groups=[list(range(NC))],
                ins=[ag2_in[:].opt()], outs=[ag2_out[:].opt()])

            hfull = ag2_out[:].rearrange("s (jc p t) -> p s jc t", jc=NDC, p=128, t=TOK)
            htl = []
            for half in range(2):
                ht = htpool.tile([128, NDC, 1024], bf16, tag="ht", name=f"ht{half}")
                for s in range(4):
                    nc.scalar.dma_start(
                        out=ht[:, :, s * TOK:(s + 1) * TOK],
                        in_=hfull[:, 4 * half + s, :, :])
                htl.append(ht)

            pwsrc = pw_d.rearrange("(dc p) v -> p dc v", p=128)
            for wcg in range(VPAD // 512):
                pwc = wpool.tile([128, NDC, 512], bf16, tag="wch")
                nc.sync.dma_start(out=pwc, in_=pwsrc[:, :, wcg * 512:(wcg + 1) * 512])
                for vc in range(4):
                    vg = wcg * 4 + vc
                    for half in range(2):
                        lsb = lpool.tile([128, 1024], f32, tag="lsb")
                        for tq in range(2):
                            ps = psmm.tile([128, 512], f32, tag="mm")
                            for dc in range(NDC):
                                nc.tensor.matmul(ps[:], pwc[:, dc, vc * 128:(vc + 1) * 128],
                                                 htl[half][:, dc, tq * 512:(tq + 1) * 512],
                                                 start=(dc == 0), stop=(dc == NDC - 1))
                            nc.vector.tensor_scalar(out=lsb[:, tq * 512:(tq + 1) * 512],
                                                    in0=ps[:],
                                                    scalar1=pbt[:, vg:vg + 1], scalar2=None,
                                                    op0=mybir.AluOpType.add)
                        nc.sync.dma_start(
                            out=out_d[vg * 128:(vg + 1) * 128,
                                      half * 1024:(half + 1) * 1024],
                            in_=lsb[:])
    nc.compile()
    return nc


def kernel(**inputs):
    x = np.asarray(inputs["x"])
    emb = np.asarray(inputs["emb"], dtype=np.float32)

    ln1_g = np.asarray(inputs["ln1_g"], dtype=np.float32)
    ln1_b = np.asarray(inputs["ln1_b"], dtype=np.float32)
    ln2_g = np.asarray(inputs["ln2_g"], dtype=np.float32)
    ln2_b = np.asarray(inputs["ln2_b"], dtype=np.float32)
    lnf_g = np.asarray(inputs["lnf_g"], dtype=np.float32)
    lnf_b = np.asarray(inputs["lnf_b"], dtype=np.float32)
    b2 = np.asarray(inputs["b2"], dtype=np.float32)

    ln1_triv = bool(np.all(ln1_g == 1) and np.all(ln1_b == 0))
    ln2_triv = bool(np.all(ln2_g == 1) and np.all(ln2_b == 0))
    lnf_triv = bool(np.all(lnf_g == 1) and np.all(lnf_b == 0))
    b2_triv = bool(np.all(b2 == 0))

    nc = build_program(ln1_triv, ln2_triv, b2_triv, lnf_triv)

    pe_full = _pe()
    ids = np.asarray(x).reshape(B * S)
    pw_full = np.asarray(inputs["projW"], dtype=np.float32)
    pb_full = np.asarray(inputs["projb"], dtype=np.float32)

    import ml_dtypes
    bfl = ml_dtypes.bfloat16
    # causal mask in [k, q] layout for the diagonal block: valid iff k <= q
    cmask = np.ascontiguousarray(
        np.triu(np.ones((128, 128), dtype=np.float32)).astype(bfl))
    shared = {
        "wq": np.ascontiguousarray(np.asarray(inputs["Wq"], dtype=bfl)),
        "wk": np.ascontiguousarray(np.asarray(inputs["Wk"], dtype=bfl)),
        "wv": np.ascontiguousarray(np.asarray(inputs["Wv"], dtype=bfl)),
        "wo": np.ascontiguousarray(np.asarray(inputs["Wo"], dtype=bfl)),
        "w1": np.ascontiguousarray(np.asarray(inputs["W1"], dtype=bfl)),
        "w2": np.ascontiguousarray(np.asarray(inputs["W2"], dtype=bfl)),
        "b1": np.ascontiguousarray(inputs["b1"], dtype=np.float32),
        "cmask": cmask,
    }
    if not (ln1_triv and ln2_triv):
        lngb = np.stack([
            np.broadcast_to(ln1_g[:, None, :], (L, 128, D)),
            np.broadcast_to(ln1_b[:, None, :], (L, 128, D)),
            np.broadcast_to(ln2_g[:, None, :], (L, 128, D)),
            np.broadcast_to(ln2_b[:, None, :], (L, 128, D)),
        ], axis=1)
        shared["lngb"] = np.ascontiguousarray(lngb, dtype=np.float32)
    if not b2_triv:
        shared["b2b"] = np.ascontiguousarray(
            np.broadcast_to(b2[:, None, :], (L, 128, D)), dtype=np.float32)
    if not lnf_triv:
        shared["fgb"] = np.ascontiguousarray(
            np.stack([np.broadcast_to(lnf_g[None, :], (128, D)),
                      np.broadcast_to(lnf_b[None, :], (128, D))]), dtype=np.float32)

    in_maps = []
    for c in range(NC):
        b = c // 4
        q0 = (c % 4) * TOK
        sl = slice(b * S + q0, b * S + q0 + TOK)
        embx = np.ascontiguousarray(emb[ids[sl]], dtype=np.float32)
        pes = np.ascontiguousarray(pe_full[q0:q0 + TOK], dtype=np.float32)
        pw = np.zeros((D, VPAD), dtype=bfl)
        pw[:, :VSH] = np.asarray(pw_full[:, c * VSH:(c + 1) * VSH], dtype=bfl)
        pb = np.zeros((VPAD,), dtype=np.float32)
        pb[:VSH] = pb_full[c * VSH:(c + 1) * VSH]
        im = dict(shared)
        im.update({"embx": embx, "pe": pes, "pw": pw, "pb": np.ascontiguousarray(pb)})
        in_maps.append(im)

    trace = bool(int(os.environ.get("KERNEL_TRACE", "0")))
    if trace:
        _install_trace_hook()
    res = run_bass_kernel_spmd(nc, in_maps, core_ids=list(range(NC)), trace=trace)
    if trace:
        kernel.last_exec_time_ns = res.exec_time_ns

    parts = [res.results[c]["logits"][:VSH, :] for c in range(NC)]
    full = np.concatenate(parts, axis=0)          # [V, B*S]
    return np.ascontiguousarray(full.T.reshape(B, S, V))


def _install_trace_hook():
    import types
    if 'antenv.axon_hooks' in sys.modules:
        return
    try:
        import trn_agent_boot.trn_boot as trn_boot
        mod = types.ModuleType('antenv.axon_hooks')
        _hook = [None]
        mod.set_axon_ntff_profile_hook = lambda hk: _hook.__setitem__(0, hk)
        mod.get_axon_ntff_profile_hook = lambda: _hook[0]
        sys.modules['antenv.axon_hooks'] = mod
        import antenv
        antenv.axon_hooks = mod
        mod.set_axon_ntff_profile_hook(
            trn_boot._ntff_profile_via_ctypes('/opt/axon/libaxon_pjrt.so'))
    except Exception as e:
        print(f"trace hook unavailable: {e}", file=sys.stderr)


# revision 14
# speedup vs baseline: 1.4557x; 1.0099x over previous
import sys, os
sys.path.insert(0, '/opt/trn_rl_repo')
import numpy as np
import concourse.bass as bass
import concourse.tile as tile
from concourse import bacc, mybir
from concourse.bass_utils import run_bass_kernel_spmd
from concourse.masks import make_identity

dt = mybir.dt
f32, f32r, bf16 = dt.float32, dt.float32r, dt.bfloat16

B, S, D, DFF, H, V, L = 2, 1024, 1024, 4096, 16, 32000, 8
DH = D // H          # 64
NC = 8               # cores
TOK = (B * S) // NC  # 256 tokens per core
NTB = TOK // 128     # 2 token blocks per core
NDC = D // 128       # 8 d-chunks
NKC = S // 128       # 8 key chunks per batch
NFFC = DFF // 128    # 32
VSH = V // NC        # 4000 vocab per core
VPAD = 4096          # padded vocab slice
EPS = 1e-5
ASCALE = 16.0        # fp8 scale for K/Q/V a2a payloads


def _pe():
    pos = np.arange(S, dtype=np.float32)[:, None]
    div = np.exp(np.arange(0, D, 2, dtype=np.float32) * (-np.log(10000.0) / D))
    pe = np.zeros((S, D), dtype=np.float32)
    pe[:, 0::2] = np.sin(pos * div)
    pe[:, 1::2] = np.cos(pos * div)
    return pe


def build_program(ln1_triv, ln2_triv, b2_triv, lnf_triv):
    nc = bacc.Bacc("TRN2", target_bir_lowering=False, debug=False,
                   enable_asserts=False, num_devices=NC)

    # ---- DRAM inputs (shared weights) ----
    wq_d = nc.dram_tensor("wq", [L, D, D], bf16, kind="ExternalInput").ap()
    wk_d = nc.dram_tensor("wk", [L, D, D], bf16, kind="ExternalInput").ap()
    wv_d = nc.dram_tensor("wv", [L, D, D], bf16, kind="ExternalInput").ap()
    wo_d = nc.dram_tensor("wo", [L, D, D], bf16, kind="ExternalInput").ap()
    f8 = dt.float8e4
    w1_d = nc.dram_tensor("w1", [L, D, DFF], bf16, kind="ExternalInput").ap()
    w2_d = nc.dram_tensor("w2", [L, DFF, D], bf16, kind="ExternalInput").ap()
    b1_d = nc.dram_tensor("b1", [L, DFF], f32, kind="ExternalInput").ap()
    # ---- per-core inputs ----
    embx_d = nc.dram_tensor("embx", [TOK, D], f32, kind="ExternalInput").ap()
    pe_d = nc.dram_tensor("pe", [TOK, D], f32, kind="ExternalInput").ap()
    # causal mask for diagonal 128x128 blocks, [k, q] layout (triu)
    cm_d = nc.dram_tensor("cmask", [128, 128], bf16, kind="ExternalInput").ap()
    pw_d = nc.dram_tensor("pw", [D, VPAD], bf16, kind="ExternalInput").ap()
    pb_d = nc.dram_tensor("pb", [VPAD], f32, kind="ExternalInput").ap()
    if not (ln1_triv and ln2_triv):
        lngb_d = nc.dram_tensor("lngb", [L, 4, 128, D], f32, kind="ExternalInput").ap()
    if not b2_triv:
        b2b_d = nc.dram_tensor("b2b", [L, 128, D], f32, kind="ExternalInput").ap()
    if not lnf_triv:
        fgb_d = nc.dram_tensor("fgb", [2, 128, D], f32, kind="ExternalInput").ap()
    # ---- output ----
    out_d = nc.dram_tensor("logits", [VPAD, B * S], f32, kind="ExternalOutput").ap()

    from contextlib import ExitStack
    with tile.TileContext(nc) as tc:
        with ExitStack() as ctx:
            cpool = ctx.enter_context(tc.tile_pool(name="const", bufs=1))
            rpool = ctx.enter_context(tc.tile_pool(name="resid", bufs=1))
            alnpool = ctx.enter_context(tc.tile_pool(name="aln", bufs=2))
            atpool = ctx.enter_context(tc.tile_pool(name="aT", bufs=2))
            kqpool = ctx.enter_context(tc.tile_pool(name="kqv", bufs=1))
            gpool = ctx.enter_context(tc.tile_pool(name="gath", bufs=1))
            a2pool = ctx.enter_context(tc.tile_pool(name="at2", bufs=2))
            wpool = ctx.enter_context(tc.tile_pool(name="wch", bufs=4))
            w2pool = ctx.enter_context(tc.tile_pool(name="w2c", bufs=4))
            upool = ctx.enter_context(tc.tile_pool(name="ut", bufs=1))
            htpool = ctx.enter_context(tc.tile_pool(name="ht", bufs=2))
            lpool = ctx.enter_context(tc.tile_pool(name="lsb", bufs=2))
            spool = ctx.enter_context(tc.tile_pool(name="small", bufs=4))
            recpool = ctx.enter_context(tc.tile_pool(name="recp", bufs=2))
            psmm = ctx.enter_context(tc.tile_pool(name="ps_mm", bufs=6, space="PSUM"))
            pso = ctx.enter_context(tc.tile_pool(name="ps_o", bufs=2, space="PSUM"))
            dpool = ctx.enter_context(tc.tile_pool(name="dram", bufs=1, space="DRAM"))

            # ---------------- constants ----------------
            ident = cpool.tile([128, 128], bf16, tag="ident")
            make_identity(nc, ident[:])
            ones = cpool.tile([128, 64], bf16, tag="ones")
            nc.vector.memset(ones[:], 1.0)
            epst = cpool.tile([128, 1], f32, tag="eps")
            nc.vector.memset(epst[:], EPS)
            nlsc = cpool.tile([128, 1], f32, tag="nlsc")
            nc.vector.memset(nlsc[:], float(-np.log(ASCALE)))
            cmt = cpool.tile([128, 128], bf16, tag="cm")
            nc.sync.dma_start(out=cmt[:], in_=cm_d)
            b1t = cpool.tile([128, L, NFFC], f32, tag="b1")
            nc.sync.dma_start(out=b1t[:], in_=b1_d.rearrange("l (fc p) -> p l fc", p=128))
            pbt = cpool.tile([128, VPAD // 128], f32, tag="pb")
            nc.sync.dma_start(out=pbt[:], in_=pb_d.rearrange("(vc p) -> p vc", p=128))
            if not (ln1_triv and ln2_triv):
                lngb = cpool.tile([128, L, 4, D], f32, tag="lngb")
                nc.sync.dma_start(out=lngb[:], in_=lngb_d.rearrange("l f p d -> p l f d"))
            if not b2_triv:
                b2b = cpool.tile([128, L, D], f32, tag="b2b")
                nc.sync.dma_start(out=b2b[:], in_=b2b_d.rearrange("l p d -> p l d"))
            if not lnf_triv:
                fgb = cpool.tile([128, 2, D], f32, tag="fgb")
                nc.sync.dma_start(out=fgb[:], in_=fgb_d.rearrange("f p d -> p f d"))

            # collective DRAM buffers (flat; chunk j = dest/src rank j)
            k_in = dpool.tile([NC * 128 * TOK], f8, tag="k_in")
            k_out = dpool.tile([NC * 128 * TOK], f8, tag="k_out")
            q_in = dpool.tile([NC * 128 * TOK], f8, tag="q_in")
            q_out = dpool.tile([NC * 128 * TOK], f8, tag="q_out")
            v_in = dpool.tile([NC * NTB * 128 * 128], f8, tag="v_in")
            v_out = dpool.tile([NC * NTB * 128 * 128], f8, tag="v_out")
            o_in = dpool.tile([NC * 128 * TOK], bf16, tag="o_in")
            o_out = dpool.tile([NC * 128 * TOK], bf16, tag="o_out")
            ag2_in = dpool.tile([TOK * D], bf16, tag="ag2_in")
            ag2_out = dpool.tile([NC, TOK * D], bf16, tag="ag2_out", addr_space="Shared")
            wu_in = dpool.tile([NC * 128], bf16, tag="wu_in")
            wu_out = dpool.tile([NC * 128], bf16, tag="wu_out")

            # tiny warmup collective: absorbs comm-stack init + start skew
            nc.gpsimd.collective_compute(
                "AllToAll", mybir.AluOpType.bypass,
                replica_groups=[list(range(NC))],
                ins=[wu_in[:].opt()], outs=[wu_out[:].opt()])

            # persistent SBUF tiles (reused across layers)
            h = [rpool.tile([128, D], f32, tag=f"h{tb}", name=f"h{tb}") for tb in range(NTB)]
            kT = kqpool.tile([128, NDC, TOK], f8, tag="kT")
            qT = kqpool.tile([128, NDC, TOK], f8, tag="qT")
            vloc = [kqpool.tile([128, D], f8, tag=f"vloc{tb}", name=f"vloc{tb}")
                    for tb in range(NTB)]
            Kg = [gpool.tile([128, S], f8, tag=f"Kg{b}", name=f"Kg{b}") for b in range(B)]
            Qg = [gpool.tile([128, S], f8, tag=f"Qg{b}", name=f"Qg{b}") for b in range(B)]
            vt = [gpool.tile([128, NKC, 130], f8, tag=f"vt{b}", name=f"vt{b}")
                  for b in range(B)]
            oTl = [gpool.tile([128, S], bf16, tag=f"oTl{b}", name=f"oTl{b}") for b in range(B)]
            oTfull = gpool.tile([128, NDC, TOK], bf16, tag="oTfull")
            # ones columns of vt (once; data DMAs each layer only touch cols 0:64/65:129)
            for b in range(B):
                nc.vector.memset(vt[b][:, :, 64:65], 1.0)
                nc.vector.memset(vt[b][:, :, 129:130], 1.0)

            # ---------------- embedding ----------------
            for tb in range(NTB):
                et = lpool.tile([128, D], f32, tag="lsb")
                pt = lpool.tile([128, D], f32, tag="lsb")
                nc.sync.dma_start(out=et, in_=embx_d[tb * 128:(tb + 1) * 128, :])
                nc.sync.dma_start(out=pt, in_=pe_d[tb * 128:(tb + 1) * 128, :])
                nc.vector.tensor_scalar(out=et[:], in0=et[:], scalar1=float(np.sqrt(D)),
                                        scalar2=None, op0=mybir.AluOpType.mult)
                nc.vector.tensor_add(h[tb][:], et[:], pt[:])

            def layer_norm(src_tiles, gb=None):
                outs = []
                for tb in range(NTB):
                    st = spool.tile([128, 2, 6], f32, tag="bnst")
                    xin = src_tiles[tb][:].rearrange("p (g d) -> p g d", g=2)
                    for g in range(2):
                        nc.vector.bn_stats(out=st[:, g, :], in_=xin[:, g, :])
                    mv = spool.tile([128, 2], f32, tag="bnmv")
                    nc.vector.bn_aggr(out=mv[:], in_=st[:])
                    std = spool.tile([128, 1], f32, tag="bnsd")
                    nc.scalar.activation(out=std[:], in_=mv[:, 1:2],
                                         func=mybir.ActivationFunctionType.Sqrt,
                                         bias=epst[:], scale=1.0)
                    nc.vector.reciprocal(out=std[:], in_=std[:])
                    at = alnpool.tile([128, D], bf16, tag="aln")
                    nc.vector.tensor_scalar(out=at[:], in0=src_tiles[tb][:],
                                            scalar1=mv[:, 0:1], scalar2=std[:],
                                            op0=mybir.AluOpType.subtract,
                                            op1=mybir.AluOpType.mult)
                    if gb is not None:
                        gt, bt = gb
                        nc.vector.tensor_mul(at[:], at[:], gt)
                        nc.vector.tensor_add(at[:], at[:], bt)
                    outs.append(at)
                return outs

            def transpose_to_aT(src_tiles):
                at = atpool.tile([128, NDC, TOK], bf16, tag="aT")
                for tb in range(NTB):
                    for dc in range(NDC):
                        ps = psmm.tile([128, 128], bf16, tag="mm")
                        nc.tensor.transpose(ps[:], src_tiles[tb][:, dc * 128:(dc + 1) * 128],
                                            ident[:])
                        nc.vector.tensor_copy(out=at[:, dc, tb * 128:(tb + 1) * 128], in_=ps[:])
                return at

            def proj_jc(wsrc_l, aT, dst):
                """dst[128, jc, TOK] = (W.T @ a.T); W [D, D] d-major chunks."""
                wsrc = wsrc_l.rearrange("(dc p) j -> p dc j", p=128)
                for cg in range(2):
                    wc = wpool.tile([128, NDC, 512], bf16, tag="wch")
                    nc.sync.dma_start(out=wc, in_=wsrc[:, :, cg * 512:(cg + 1) * 512])
                    for oc in range(4):
                        jc = cg * 4 + oc
                        ps = psmm.tile([128, TOK], f32, tag="mm")
                        for dc in range(NDC):
                            nc.tensor.matmul(ps[:], wc[:, dc, oc * 128:(oc + 1) * 128],
                                             aT[:, dc, :], start=(dc == 0), stop=(dc == NDC - 1))
                        with nc.allow_low_precision(reason="fp8 a2a payload"):
                            nc.vector.tensor_scalar(out=dst[:, jc, :], in0=ps[:],
                                                    scalar1=ASCALE, scalar2=None,
                                                    op0=mybir.AluOpType.mult)

            # views of the collective buffers
            k_in_v = k_in[:].rearrange("(j p k) -> p j k", j=NC, p=128)
            k_out_v = k_out[:].rearrange("(s p k) -> p s k", s=NC, p=128)
            q_in_v = q_in[:].rearrange("(j p k) -> p j k", j=NC, p=128)
            q_out_v = q_out[:].rearrange("(s p k) -> p s k", s=NC, p=128)
            v_in_v = v_in[:].rearrange("(j tb p e) -> p j tb e", j=NC, tb=NTB, p=128)
            v_out_v = v_out[:].rearrange("(s tb p e) -> p s tb e", s=NC, tb=NTB, p=128)
            o_in_v = o_in[:].rearrange("(j p k) -> p j k", j=NC, p=128)
            o_out_v = o_out[:].rearrange("(s p k) -> p s k", s=NC, p=128)

            # ---------------- layers ----------------
            for li in range(L):
                ln1_gb = None
                if not ln1_triv:
                    ln1_gb = (lngb[:, li, 0, :], lngb[:, li, 1, :])
                aln = layer_norm(h, ln1_gb)
                aT = transpose_to_aT(aln)

                # K -> A2A_K; Q -> A2A_Q (each overlaps the next projection)
                proj_jc(wk_d[li], aT, kT)
                nc.sync.dma_start(out=k_in_v, in_=kT[:])
                nc.gpsimd.collective_compute(
                    "AllToAll", mybir.AluOpType.bypass,
                    replica_groups=[list(range(NC))],
                    ins=[k_in[:].opt()], outs=[k_out[:].opt()])
                proj_jc(wq_d[li], aT, qT)
                nc.sync.dma_start(out=q_in_v, in_=qT[:])
                nc.gpsimd.collective_compute(
                    "AllToAll", mybir.AluOpType.bypass,
                    replica_groups=[list(range(NC))],
                    ins=[q_in[:].opt()], outs=[q_out[:].opt()])

                # V projection (token-major) overlaps the KQ A2A
                wsrc = wv_d[li].rearrange("(dc p) j -> p dc j", p=128)
                for cg in range(2):
                    wc = wpool.tile([128, NDC, 512], bf16, tag="wch")
                    nc.sync.dma_start(out=wc, in_=wsrc[:, :, cg * 512:(cg + 1) * 512])
                    for tb in range(NTB):
                        ps = psmm.tile([128, 512], f32, tag="mm")
                        for dc in range(NDC):
                            nc.tensor.matmul(ps[:], aT[:, dc, tb * 128:(tb + 1) * 128],
                                             wc[:, dc, :], start=(dc == 0), stop=(dc == NDC - 1))
                        with nc.allow_low_precision(reason="fp8 a2a payload"):
                            nc.vector.tensor_scalar(
                                out=vloc[tb][:, cg * 512:(cg + 1) * 512], in0=ps[:],
                                scalar1=ASCALE, scalar2=None,
                                op0=mybir.AluOpType.mult)
                for tb in range(NTB):
                    nc.sync.dma_start(
                        out=v_in_v[:, :, tb, :],
                        in_=vloc[tb][:].rearrange("p (j e) -> p j e", j=NC))
                nc.gpsimd.collective_compute(
                    "AllToAll", mybir.AluOpType.bypass,
                    replica_groups=[list(range(NC))],
                    ins=[v_in[:].opt()], outs=[v_out[:].opt()])

                # unpack gathered K/Q/V per batch
                for b in range(B):
                    nc.scalar.dma_start(
                        out=Kg[b][:].rearrange("p (s k) -> p s k", s=4),
                        in_=k_out_v[:, 4 * b:4 * b + 4, :])
                    nc.scalar.dma_start(
                        out=Qg[b][:].rearrange("p (s k) -> p s k", s=4),
                        in_=q_out_v[:, 4 * b:4 * b + 4, :])
                    vtv = vt[b][:].rearrange("p (g tb) c -> p g tb c", g=4)
                    nc.scalar.dma_start(out=vtv[:, :, :, 0:64],
                                      in_=v_out_v[:, 4 * b:4 * b + 4, :, 0:64])
                    nc.scalar.dma_start(out=vtv[:, :, :, 65:129],
                                      in_=v_out_v[:, 4 * b:4 * b + 4, :, 64:128])

                # ---- attention: 2 heads x 2 batches, causal ----
                for b in range(B):
                    at2 = [a2pool.tile([128, NKC, S], bf16, tag="at2", name=f"at2_{li}_{b}_{hh}")
                           for hh in range(2)]
                    for hh in range(2):
                        for kc in (1, 2, 3):
                            nc.vector.memset(at2[hh][:, kc, 0:kc * 128], 0.0)
                        for kc in (5, 6, 7):
                            nc.vector.memset(at2[hh][:, kc, 512:kc * 128], 0.0)
                    for kc in range(NKC):
                        q0 = kc * 128
                        spans = [(q0, 512), (512, S)] if q0 < 512 else [(q0, S)]
                        for (qa, qb) in spans:
                            n = qb - qa
                            for hh in range(2):
                                hb = hh * 64
                                ps = psmm.tile([128, 512], f32, tag="mm")
                                nc.tensor.matmul(
                                    ps[:, 0:n],
                                    Kg[b][hb:hb + 64, q0:q0 + 128],
                                    Qg[b][hb:hb + 64, qa:qb], start=True, stop=True)
                                nc.scalar.activation(
                                    out=at2[hh][:, kc, qa:qb], in_=ps[:, 0:n],
                                    func=mybir.ActivationFunctionType.Exp,
                                    scale=float(1.0 / (ASCALE * ASCALE * np.sqrt(DH))))
                        for hh in range(2):
                            nc.vector.tensor_mul(at2[hh][:, kc, q0:q0 + 128],
                                                 at2[hh][:, kc, q0:q0 + 128], cmt[:])
                    for hh in range(2):
                        hb = hh * 64
                        for qc in range(2):
                            kcs = range(0, 4) if qc == 0 else range(0, 8)
                            ps_o = pso.tile([65, 512], f32, tag="o")
                            last = max(kcs)
                            for kc in kcs:
                                nc.tensor.matmul(
                                    ps_o[:], vt[b][:, kc, hh * 65:hh * 65 + 65],
                                    at2[hh][:, kc, qc * 512:(qc + 1) * 512],
                                    start=(kc == 0), stop=(kc == last))
                            ssb = recpool.tile([1, 512], bf16, tag="ssb")
                            with nc.allow_low_precision(reason="softmax denom"):
                                nc.vector.tensor_copy(out=ssb[:], in_=ps_o[64:65, :])
                            ps_rb = psmm.tile([64, 512], f32, tag="mm")
                            nc.tensor.matmul(ps_rb[:], ones[0:1, 0:64], ssb[:],
                                             start=True, stop=True)
                            lnt = recpool.tile([64, 512], f32, tag="lnt")
                            nc.scalar.activation(out=lnt[:], in_=ps_rb[:],
                                                 func=mybir.ActivationFunctionType.Ln,
                                                 scale=1.0)
                            rbs = recpool.tile([64, 512], bf16, tag="rbs")
                            nc.scalar.activation(out=rbs[:], in_=lnt[:],
                                                 func=mybir.ActivationFunctionType.Exp,
                                                 scale=-1.0, bias=nlsc[0:64, :])
                            nc.vector.tensor_mul(oTl[b][hb:hb + 64, qc * 512:(qc + 1) * 512],
                                                 ps_o[0:64, :], rbs[:])

                # ---- A2A o back to token-sharded, then Wo + residual ----
                for b in range(B):
                    nc.sync.dma_start(
                        out=o_in_v[:, 4 * b:4 * b + 4, :],
                        in_=oTl[b][:].rearrange("p (s k) -> p s k", s=4))
                nc.gpsimd.collective_compute(
                    "AllToAll", mybir.AluOpType.bypass,
                    replica_groups=[list(range(NC))],
                    ins=[o_in[:].opt()], outs=[o_out[:].opt()])
                nc.scalar.dma_start(out=oTfull[:], in_=o_out_v)

                wsrc = wo_d[li].rearrange("(jc p) o -> p jc o", p=128)
                for cg in range(2):
                    wc = wpool.tile([128, NDC, 512], bf16, tag="wch")
                    nc.sync.dma_start(out=wc, in_=wsrc[:, :, cg * 512:(cg + 1) * 512])
                    for tb in range(NTB):
                        ps = psmm.tile([128, 512], f32, tag="mm")
                        for jc in range(NDC):
                            nc.tensor.matmul(ps[:], oTfull[:, jc, tb * 128:(tb + 1) * 128],
                                             wc[:, jc, :], start=(jc == 0), stop=(jc == NDC - 1))
                        nc.vector.tensor_add(h[tb][:, cg * 512:(cg + 1) * 512],
                                             h[tb][:, cg * 512:(cg + 1) * 512], ps[:])

                # ---- FFN ----
                ln2_gb = None
                if not ln2_triv:
                    ln2_gb = (lngb[:, li, 2, :], lngb[:, li, 3, :])
                f_ln = layer_norm(h, ln2_gb)
                fT = transpose_to_aT(f_ln)

                uts = upool.tile([128, NFFC, TOK], bf16, tag="ut")
                wsrc = w1_d[li].rearrange("(dc p) j -> p dc j", p=128)
                for cg in range(NFFC // 4):
                    wc = wpool.tile([128, NDC, 512], bf16, tag="wch")
                    nc.sync.dma_start(out=wc, in_=wsrc[:, :, cg * 512:(cg + 1) * 512])
                    for oc in range(4):
                        fc = cg * 4 + oc
                        ps = psmm.tile([128, TOK], f32, tag="mm")
                        for dc in range(NDC):
                            nc.tensor.matmul(ps[:], wc[:, dc, oc * 128:(oc + 1) * 128],
                                             fT[:, dc, :], start=(dc == 0), stop=(dc == NDC - 1))
                        nc.scalar.activation(out=uts[:, fc, :], in_=ps[:],
                                             func=mybir.ActivationFunctionType.Relu,
                                             bias=b1t[:, li, fc:fc + 1], scale=1.0)

                wsrc = w2_d[li].rearrange("(fc p) o -> p fc o", p=128)
                chains = {}
                for tb in range(NTB):
                    for og in range(2):
                        chains[(tb, og)] = psmm.tile([128, 512], f32, tag="mm",
                                                     name=f"w2ch{li}{tb}{og}")
                for fcg in range(NFFC // 2):
                    wc = w2pool.tile([128, 2, D], bf16, tag="w2c")
                    nc.sync.dma_start(out=wc, in_=wsrc[:, fcg * 2:fcg * 2 + 2, :])
                    for f2 in range(2):
                        fc = fcg * 2 + f2
                        for tb in range(NTB):
                            for og in range(2):
                                nc.tensor.matmul(chains[(tb, og)][:],
                                                 uts[:, fc, tb * 128:(tb + 1) * 128],
                                                 wc[:, f2, og * 512:(og + 1) * 512],
                                                 start=(fc == 0), stop=(fc == NFFC - 1))
                for tb in range(NTB):
                    for og in range(2):
                        nc.vector.tensor_add(h[tb][:, og * 512:(og + 1) * 512],
                                             h[tb][:, og * 512:(og + 1) * 512],
                                             chains[(tb, og)][:])
                    if not b2_triv:
                        nc.vector.tensor_add(h[tb][:], h[tb][:], b2b[:, li, :])

            # ---------------- final LN + AllGather + projection ----------------
            fin_gb = None if lnf_triv else (fgb[:, 0, :], fgb[:, 1, :])
            fin = layer_norm(h, fin_gb)
            finT = transpose_to_aT(fin)
            nc.sync.dma_start(
                out=ag2_in[:].rearrange("(jc p t) -> p jc t", p=128, t=TOK),
                in_=finT[:])
            nc.gpsimd.collective_compute(
                "AllGather", mybir.AluOpType.bypass,
                replica_groups=[list(range(NC))],
                ins=[ag2_in[:].opt()], outs=[ag2_out[:].opt()])

            hfull = ag2_out[:].rearrange("s (jc p t) -> p s jc t", jc=NDC, p=128, t=TOK)
            htl = []
            for half in range(2):
                ht = htpool.tile([128, NDC, 1024], bf16, tag="ht", name=f"ht{half}")
                for s in range(4):
                    nc.scalar.dma_start(
                        out=ht[:, :, s * TOK:(s + 1) * TOK],
                        in_=hfull[:, 4 * half + s, :, :])
                htl.append(ht)

            pwsrc = pw_d.rearrange("(dc p) v -> p dc v", p=128)
            for wcg in range(VPAD // 512):
                pwc = wpool.tile([128, NDC, 512], bf16, tag="wch")
                nc.sync.dma_start(out=pwc, in_=pwsrc[:, :, wcg * 512:(wcg + 1) * 512])
                for vc in range(4):
                    vg = wcg * 4 + vc
                    for half in range(2):
                        lsb = lpool.tile([128, 1024], f32, tag="lsb")
                        for tq in range(2):
                            ps = psmm.tile([128, 512], f32, tag="mm")
                            for dc in range(NDC):
                                nc.tensor.matmul(ps[:], pwc[:, dc, vc * 128:(vc + 1) * 128],
                                                 htl[half][:, dc, tq * 512:(tq + 1) * 512],
                                                 start=(dc == 0), stop=(dc == NDC - 1))
                            nc.vector.tensor_scalar(out=lsb[:, tq * 512:(tq + 1) * 512],
                                                    in0=ps[:],
                                                    scalar1=pbt[:, vg:vg + 1], scalar2=None,
                                                    op0=mybir.AluOpType.add)
                        nc.sync.dma_start(
                            out=out_d[vg * 128:(vg + 1) * 128,
                                      half * 1024:(half + 1) * 1024],
                            in_=lsb[:])
    nc.compile()
    return nc


def kernel(**inputs):
    x = np.asarray(inputs["x"])
    emb = np.asarray(inputs["emb"], dtype=np.float32)

    ln1_g = np.asarray(inputs["ln1_g"], dtype=np.float32)
    ln1_b = np.asarray(inputs["ln1_b"], dtype=np.float32)
    ln2_g = np.asarray(inputs["ln2_g"], dtype=np.float32)
    ln2_b = np.asarray(inputs["ln2_b"], dtype=np.float32)
    lnf_g = np.asarray(inputs["lnf_g"], dtype=np.float32)
    lnf_b = np.asarray(inputs["lnf_b"], dtype=np.float32)
    b2 = np.asarray(inputs["b2"], dtype=np.float32)

    ln1_triv = bool(np.all(ln1_g == 1) and np.all(ln1_b == 0))
    ln2_triv = bool(np.all(ln2_g == 1) and np.all(ln2_b == 0))
    lnf_triv = bool(np.all(lnf_g == 1) and np.all(lnf_b == 0))
    b2_triv = bool(np.all(b2 == 0))

    nc = build_program(ln1_triv, ln2_triv, b2_triv, lnf_triv)

    pe_full = _pe()
    ids = np.asarray(x).reshape(B * S)
    pw_full = np.asarray(inputs["projW"], dtype=np.float32)
    pb_full = np.asarray(inputs["projb"], dtype=np.float32)

    import ml_dtypes
    bfl = ml_dtypes.bfloat16
    # causal mask in [k, q] layout for the diagonal block: valid iff k <= q
    cmask = np.ascontiguousarray(
        np.triu(np.ones((128, 128), dtype=np.float32)).astype(bfl))
    shared = {
        "wq": np.ascontiguousarray(np.asarray(inputs["Wq"], dtype=bfl)),
        "wk": np.ascontiguousarray(np.asarray(inputs["Wk"], dtype=bfl)),
        "wv": np.ascontiguousarray(np.asarray(inputs["Wv"], dtype=bfl)),
        "wo": np.ascontiguousarray(np.asarray(inputs["Wo"], dtype=bfl)),
        "w1": np.ascontiguousarray(np.asarray(inputs["W1"], dtype=bfl)),
        "w2": np.ascontiguousarray(np.asarray(inputs["W2"], dtype=bfl)),
        "b1": np.ascontiguousarray(inputs["b1"], dtype=np.float32),
        "cmask": cmask,
    }
    if not (ln1_triv and ln2_triv):
        lngb = np.stack([
            np.broadcast_to(ln1_g[:, None, :], (L, 128, D)),
            np.broadcast_to(ln1_b[:, None, :], (L, 128, D)),
            np.broadcast_to(ln2_g[:, None, :], (L, 128, D)),
            np.broadcast_to(ln2_b[:, None, :], (L, 128, D)),
        ], axis=1)
        shared["lngb"] = np.ascontiguousarray(lngb, dtype=np.float32)
    if not b2_triv:
        shared["b2b"] = np.ascontiguousarray(
            np.broadcast_to(b2[:, None, :], (L, 128, D)), dtype=np.float32)
    if not lnf_triv:
        shared["fgb"] = np.ascontiguousarray(
            np.stack([np.broadcast_to(lnf_g[None, :], (128, D)),
                      np.broadcast_to(lnf_b[None, :], (128, D))]), dtype=np.float32)

    in_maps = []
    for c in range(NC):
        b = c // 4
        q0 = (c % 4) * TOK
        sl = slice(b * S + q0, b * S + q0 + TOK)
        embx = np.ascontiguousarray(emb[ids[sl]], dtype=np.float32)
        pes = np.ascontiguousarray(pe_full[q0:q0 + TOK], dtype=np.float32)
        pw = np.zeros((D, VPAD), dtype=bfl)
        pw[:, :VSH] = np.asarray(pw_full[:, c * VSH:(c + 1) * VSH], dtype=bfl)
        pb = np.zeros((VPAD,), dtype=np.float32)
        pb[:VSH] = pb_full[c * VSH:(c + 1) * VSH]
        im = dict(shared)
        im.update({"embx": embx, "pe": pes, "pw": pw, "pb": np.ascontiguousarray(pb)})
        in_maps.append(im)

    trace = bool(int(os.environ.get("KERNEL_TRACE", "0")))
    if trace:
        _install_trace_hook()
    res = run_bass_kernel_spmd(nc, in_maps, core_ids=list(range(NC)), trace=trace)
    if trace:
        kernel.last_exec_time_ns = res.exec_time_ns

    parts = [res.results[c]["logits"][:VSH, :] for c in range(NC)]
    full = np.concatenate(parts, axis=0)          # [V, B*S]
    return np.ascontiguousarray(full.T.reshape(B, S, V))


def _install_trace_hook():
    import types
    if 'antenv.axon_hooks' in sys.modules:
        return
    try:
        import trn_agent_boot.trn_boot as trn_boot
        mod = types.ModuleType('antenv.axon_hooks')
        _hook = [None]
        mod.set_axon_ntff_profile_hook = lambda hk: _hook.__setitem__(0, hk)
        mod.get_axon_ntff_profile_hook = lambda: _hook[0]
        sys.modules['antenv.axon_hooks'] = mod
        import antenv
        antenv.axon_hooks = mod
        mod.set_axon_ntff_profile_hook(
            trn_boot._ntff_profile_via_ctypes('/opt/axon/libaxon_pjrt.so'))
    except Exception as e:
        print(f"trace hook unavailable: {e}", file=sys.stderr)
